# revision 36
# baseline (speedup 1.0000x reference)
"""GQA (B=2, L=2048, D=2048, H=16, KVH=4, HD=128) on 8 Trainium2 NeuronCores.

Sharding: core c = (batch b = c//4, kv-group g = c%4). Each core computes its
group's 4 query heads + 1 KV head end-to-end and a partial output projection
(Wo in-dim slice); the host sums the 4 partials per batch (tensor-parallel
unshard) -- no on-device collectives.

Structure (236us vs 348us baseline; phase A 98% / phase B+C 95% PE busy):
  - All inputs host-packed into ONE contiguous [128, 57472] bf16 DRAM tensor
    per core, ordered by first use (small wk/wv first so K/V projections run
    while the big wq block streams); ~9 fat DMA descriptors replace 185
    small ones, collapsing the 56us DMA-only prologue.
  - Rope tables shared between Q and K (attention scale folded into Wq on
    host); rope eviction split ScalarE (partition-swap copies) / VectorE
    (mults+add, cos-mult reads PSUM directly).
  - V projection accumulates 4 l-blocks into one [128,512] PSUM tile with a
    single eviction into the packed V buffer (already in [l, hd] layout).
  - Phase B is chunk-outer/head-inner. Row-sums come off the PE: VectorE
    accumulates P.T tiles in SBUF, one ones-matmul per (head,chunk) does the
    partition reduction (saves ~31us of PE streaming).
  - Causal subrange AV matmuls (no masked-region streaming, no gpsimd
    memsets; pt garbage regions are simply never read). One exp per S-pair
    (masked gaps exp'd but never read).
  - Chunk-0 attention (ACT-heavy, nothing to overlap with) is interleaved
    piecewise between phase-A projection chains on dedicated PSUM pools.
  - Output projection for chunk c-1 is interleaved between phase-B head
    iterations of chunk c, emitted BEFORE each rowsum matmul so the in-order
    PE queue has ready work while the DVE acc chain catches up.
  - bf16 output (halves the output DMA; host gathers in fp32).
Tried and rejected: fp8e4 DoubleRow out-projection (4e-2 rel err - over the
2e-2 gate - and slower: 256-col LDWEIGHTS doesn't pipeline under DR MMs).
"""

import re
from contextlib import ExitStack

import ml_dtypes
import numpy as np

import concourse.bass as bass
import concourse.tile as tile
from concourse import mybir
from concourse.bass_utils import run_bass_kernel_spmd
from bass_rust import ScopedClock, VectorClock

dt = mybir.dt
BF16 = ml_dtypes.bfloat16
FP8 = ml_dtypes.float8_e4m3   # TRN float8e4 (IEEE e4m3, max 240)

# fp8e4 DoubleRow output projection: measured 4.0e-2 rel err (over the 2e-2
# gate; DVE fp32->fp8 cast noise is ~2x the round-to-nearest estimate) AND
# slower (285us vs 245us: DR matmuls don't pipeline with their 256-col
# LDWEIGHTS here). Keep False.
FP8_WO = False

B, L, D = 2, 2048, 2048
H, KVH, HD = 16, 4, 128
G = H // KVH          # 4 query heads per kv head (= per core)
GD = G * HD           # 512: per-core q-head feature dim
THETA = 10000.0
SCALE = HD ** -0.5
NLT = L // 128        # 16 l-tiles
NDT = D // 128        # 16 d-tiles
NLC = L // 512        # 4 l-chunks

# packed input column offsets (bf16 columns of the [128, NCOLS] input),
# ordered by first use: the small K/V weights land first so the K/V
# projections run while the big wq block is still streaming in
OWK = 0                   # 16 tiles x [128, 128]
OWV = OWK + NDT * 128     # 16 tiles x [128, 128]
OX0 = OWV + NDT * 128     # x chunk 0: 16 tiles x [128, 512]
OCOS = OX0 + NDT * 512    # [128, 2048]
OSIN = OCOS + L           # [128, 2048]
OTRI = OSIN + L           # [128, 128]
OWQ = OTRI + 128          # 16 tiles x [128, 512]  (wq, scale folded in)
OX123 = OWQ + NDT * 512   # x chunks 1-3: 3 x 16 tiles x [128, 512]
OWO = OX123 + 3 * NDT * 512  # 4 tiles x [128, 2048]
NCOLS = OWO + G * D


def _patch_tile_drain():
    """walrus in this container rejects multi-wait instructions on the SP
    queue; split the TileContext exit drain into one drain per proc."""
    def _drain_and_barrier_split(self, tick_clock, wait_clock):
        ticks = [int(s) for s in re.findall(r"\d+", str(tick_clock.global_clock))]
        for proc, t in enumerate(ticks):
            if t <= 0:
                continue
            vc = VectorClock()
            vc.require_at_least(proc, t)
            d = self.nc.sync.drain()
            wait_clock.add_sem_waits(d.ins, ScopedClock({None: vc}))
        self.nc.all_engine_barrier()
        assert self.sems is not None
        popped = self.nc._tile_sem_poison_stack.pop()
        assert popped is self._sem_poison
        self.nc.clear_and_free_semaphores(list(self.sems.allocated().values()))
        self.nc.all_engine_barrier()

    tile.TileContext._drain_and_barrier = _drain_and_barrier_split


def _split_multi_waits(nc):
    """This walrus build supports one sem-wait command per instruction; hoist
    excess waits onto same-engine NoOps inserted immediately before."""
    uid = 0
    for fn in nc.m.functions:
        for bb in fn.blocks:
            out = []
            for inst in bb.instructions:
                si = inst.sync_info
                if si is not None and si.on_wait and len(si.on_wait) > 1:
                    for w in si.on_wait[:-1]:
                        nop = mybir.InstNoOp(name=f"waitsplit-{uid}", ins=[], outs=[])
                        uid += 1
                        nop.engine = inst.engine
                        nop.sync_info = mybir.SyncInfo(on_wait=[w], on_update=[])
                        out.append(nop)
                    inst.sync_info = mybir.SyncInfo(
                        on_wait=[si.on_wait[-1]], on_update=si.on_update)
                out.append(inst)
            bb.instructions[:] = out


def _build_program():
    _patch_tile_drain()
    nc = bass.Bass("TRN2", target_bir_lowering=False, debug=False)

    inpack = nc.dram_tensor("inpack", [128, NCOLS], dt.bfloat16, kind="ExternalInput").ap()
    outT = nc.dram_tensor("outT", [D, L], dt.bfloat16, kind="ExternalOutput").ap()

    with tile.TileContext(nc) as tc:
        with ExitStack() as ctx:
            persist = ctx.enter_context(tc.tile_pool(name="persist", bufs=1))

            # --- persistent SBUF residents ---
            wkv = persist.tile([128, 2 * NDT * 128], dt.bfloat16, tag="wkv", name="wkv")
            trig = persist.tile([128, 2 * L + 128], dt.bfloat16, tag="trig", name="trig")
            wqpack = persist.tile([128, NDT * 512], dt.bfloat16, tag="wqpack", name="wqpack")
            xpack = persist.tile([128, NLC * NDT * 512], dt.bfloat16, tag="xpack", name="xpack")
            wo_dt = dt.float8e4 if FP8_WO else dt.bfloat16
            wo_cols = G * D // 2 if FP8_WO else G * D
            wopack = persist.tile([128, wo_cols * (2 if FP8_WO else 1)], wo_dt,
                                  tag="wopack", name="wopack")
            ones_sb = persist.tile([128, 128], dt.bfloat16, tag="ones", name="ones")
            qt_sb = [persist.tile([HD, L], dt.bfloat16, tag=f"qt{h}", name=f"qt{h}") for h in range(G)]
            kt_sb = persist.tile([HD, L], dt.bfloat16, tag="kt", name="kt")
            vpack = persist.tile([128, L], dt.bfloat16, tag="vpack", name="vpack")
            if FP8_WO:
                # two head-pair tiles: [head 2*g2 | head 2*g2+1] along columns
                otp = [persist.tile([128, 2 * L], dt.float8e4, tag=f"otp{g2}", name=f"otp{g2}")
                       for g2 in range(2)]
            else:
                ot_sb = [persist.tile([HD, L], dt.bfloat16, tag=f"ot{h}", name=f"ot{h}")
                         for h in range(G)]

            # --- input DMAs, ordered by first use (single queue runs them
            # FIFO): wk/wv -> x0 (2 halves) -> rope/tri -> wq -> x1..x3 -> wo
            nc.sync.dma_start(out=wkv, in_=inpack[:, OWK:OX0])
            for q in range(4):      # x0 in quarters: smoother chain pacing
                nc.sync.dma_start(
                    out=xpack[:, q * 2048:(q + 1) * 2048],
                    in_=inpack[:, OX0 + q * 2048:OX0 + (q + 1) * 2048])
            nc.sync.dma_start(out=trig, in_=inpack[:, OCOS:OWQ])
            nc.sync.dma_start(out=wqpack, in_=inpack[:, OWQ:OX123])
            for lc in range(1, NLC):
                nc.sync.dma_start(
                    out=xpack[:, lc * 8192:(lc + 1) * 8192],
                    in_=inpack[:, OX123 + (lc - 1) * 8192:OX123 + lc * 8192])
            if FP8_WO:
                # wo8 bytes ride in the bf16 inpack (half the columns)
                nc.sync.dma_start(
                    out=wopack, in_=inpack[:, OWO:OWO + G * D // 2].bitcast(dt.float8e4))
            else:
                nc.sync.dma_start(out=wopack, in_=inpack[:, OWO:NCOLS])
            nc.vector.memset(ones_sb, 1.0)

            cos_sb = trig[:, 0:L]
            sin_sb = trig[:, L:2 * L]
            tri_sb = trig[:, 2 * L:2 * L + 128]
            wk_off = 0            # wk tiles at wkv[:, i*128:...]
            wv_off = OWV - OWK

            def xsl(lc, i):          # moving x tile [128, 512]
                return xpack[:, lc * 8192 + i * 512: lc * 8192 + (i + 1) * 512]

            # SBUF working pools shared by phase A-embedded B0 and phase B/C
            ptp = ctx.enter_context(tc.tile_pool(name="pt", bufs=3))
            accp = ctx.enter_context(tc.tile_pool(name="acc", bufs=2))
            smp = ctx.enter_context(tc.tile_pool(name="sm", bufs=2))
            evp = ctx.enter_context(tc.tile_pool(name="ev", bufs=6))

            # ---- shared attention emit helpers (used for c=0 inside phase A
            # and c=1..3 in the main loop) ----
            def emit_pair(c, bi, qs, psS_p):
                jts = [2 * bi, 2 * bi + 1]
                offs = [(jt - 4 * c) * 128 if jt >= 4 * c else 0 for jt in jts]
                ps = psS_p.tile([128, 1024], dt.float32, tag="psS", name="ps")
                pt = ptp.tile([128, 1024], dt.bfloat16, tag="pt", name="pt")
                for k, (jt, off) in enumerate(zip(jts, offs)):
                    nc.tensor.matmul(
                        ps[:, k * 512 + off:(k + 1) * 512],
                        kt_sb[:, jt * 128:(jt + 1) * 128],
                        qs[:, off:], start=True, stop=True)
                # one exp per pair; masked gap regions get exp'd too (pairs
                # never straddle the diagonal: 4c is even) but are never read
                nc.scalar.activation(
                    pt[:, offs[0]:], ps[:, offs[0]:],
                    mybir.ActivationFunctionType.Exp)
                for k, (jt, off) in enumerate(zip(jts, offs)):
                    if jt >= 4 * c:
                        blk = pt[:, k * 512 + off:k * 512 + off + 128]
                        nc.vector.tensor_tensor(blk, blk, tri_sb, mybir.AluOpType.mult)
                return pt, jts, offs

            def emit_av(c, bi, njt, pair, po, acc):
                pt, jts, offs = pair
                last_bi = bi == njt // 2 - 1
                for k, (jt, off) in enumerate(zip(jts, offs)):
                    pk = pt[:, k * 512 + off:(k + 1) * 512]
                    nc.tensor.matmul(
                        po[:, off:], vpack[:, jt * 128:(jt + 1) * 128], pk,
                        start=(bi == 0 and k == 0), stop=(last_bi and k == 1))
                    if bi == 0 and k == 0:
                        nc.vector.tensor_copy(acc, pk)
                    else:
                        nc.vector.tensor_tensor(
                            acc[:, off:], acc[:, off:], pk, mybir.AluOpType.add)

            def emit_fin(c, h, po, acc, psX_p):
                pr = psX_p.tile([128, 512], dt.float32, tag="psX", name="pr")
                nc.tensor.matmul(pr, ones_sb, acc, start=True, stop=True)
                lnr = smp.tile([128, 512], dt.float32, tag="lnr", name="lnr")
                nc.scalar.activation(lnr, pr, mybir.ActivationFunctionType.Ln)
                rcp = smp.tile([128, 512], dt.float32, tag="rcp", name="rcp")
                nc.scalar.activation(rcp, lnr, mybir.ActivationFunctionType.Exp, scale=-1.0)
                if FP8_WO:
                    odst = otp[h // 2][:, (h % 2) * L + c * 512:(h % 2) * L + (c + 1) * 512]
                else:
                    odst = ot_sb[h][:, c * 512:(c + 1) * 512]
                nc.vector.tensor_tensor(odst, po, rcp, mybir.AluOpType.mult)

            # ---------------- Phase A: projections + rope, with chunk-0
            # attention interleaved piecewise between projection chains ----
            with ExitStack() as ctxA:
                psA = ctxA.enter_context(tc.tile_pool(name="psA", bufs=2, space="PSUM"))
                psB0s = ctxA.enter_context(tc.tile_pool(name="psB0s", bufs=1, space="PSUM"))
                psB0o = ctxA.enter_context(tc.tile_pool(name="psB0o", bufs=1, space="PSUM"))
                psB0x = ctxA.enter_context(tc.tile_pool(name="psB0x", bufs=1, space="PSUM"))
                ropep = ctxA.enter_context(tc.tile_pool(name="rope", bufs=2))

                # HAM warmup while input DMA streams: one dense burst of
                # dummy matmuls (no deps -> runs from t~0) sized to end just
                # before the first real MMs (~15.5us), so the PE clock gate is
                # at 8/8 when they arrive and the idle gap stays under the
                # ~3.4us re-throttle window. No trailing keep-alive chain:
                # anything after the burst sits ahead of real work in the
                # in-order PE queue and paces it (measured +6.6us).
                warm = ropep.tile([64, 128], dt.bfloat16, tag="warm", name="warm")
                nc.vector.memset(warm, 0.001)
                warm_ps = psB0x.tile([64, 128], dt.float32, tag="psX", name="pwm")
                for i in range(230):    # ~32 cold MMs @107ns + ~198 warm @56ns
                    nc.tensor.matmul(warm_ps, warm[:, 0:64], warm,
                                     start=(i == 0), stop=(i == 229))

                def rope_evict(ps, dst_slice, lc):
                    cs = cos_sb[:, lc * 512:(lc + 1) * 512]
                    sn = sin_sb[:, lc * 512:(lc + 1) * 512]
                    swp = ropep.tile([128, 512], dt.bfloat16, tag="swp", name="swp")
                    nc.scalar.copy(swp[0:64, :], ps[64:128, :])
                    nc.scalar.copy(swp[64:128, :], ps[0:64, :])
                    t1 = ropep.tile([128, 512], dt.bfloat16, tag="t1", name="t1")
                    t2 = ropep.tile([128, 512], dt.bfloat16, tag="t2", name="t2")
                    nc.vector.tensor_tensor(t1, swp, sn, mybir.AluOpType.mult)
                    nc.vector.tensor_tensor(t2, ps, cs, mybir.AluOpType.mult)
                    nc.vector.tensor_tensor(dst_slice, t1, t2, mybir.AluOpType.add)

                def b0_head_gen(h):
                    """Chunk-0 attention for head h in 3 pieces (yield = piece
                    boundary); runs on A-phase PSUM pools."""
                    qs = qt_sb[h][:, 0:512]
                    po = psB0o.tile([128, 512], dt.float32, tag="psO", name="po")
                    acc = accp.tile([128, 512], dt.bfloat16, tag="acc", name="acc")
                    pairs = []
                    for bi in range(2):
                        pairs.append(emit_pair(0, bi, qs, psB0s))
                        yield
                        emit_av(0, bi, 4, pairs[bi], po, acc)
                    emit_fin(0, h, po, acc, psB0x)

                def _step(g):
                    try:
                        next(g)
                    except StopIteration:
                        pass

                pending = []

                def pump():
                    if pending:
                        pending.pop(0)()

                def q_chain(lc, ot):
                    ps = psA.tile([128, 512], dt.float32, tag="psA", name="psA")
                    for i in range(NDT):
                        nc.tensor.matmul(
                            ps, wqpack[:, i * 512 + ot * 128:i * 512 + (ot + 1) * 128],
                            xsl(lc, i), start=(i == 0), stop=(i == NDT - 1))
                    rope_evict(ps, qt_sb[ot][:, lc * 512:(lc + 1) * 512], lc)

                def k_chain(lc):
                    ps = psA.tile([128, 512], dt.float32, tag="psA", name="psA")
                    for i in range(NDT):
                        nc.tensor.matmul(
                            ps, wkv[:, wk_off + i * 128:wk_off + (i + 1) * 128],
                            xsl(lc, i), start=(i == 0), stop=(i == NDT - 1))
                    rope_evict(ps, kt_sb[:, lc * 512:(lc + 1) * 512], lc)

                def v_chain(lc):
                    pv = psA.tile([128, 512], dt.float32, tag="psV", bufs=2, name="psV")
                    for ls in range(4):
                        for i in range(NDT):
                            nc.tensor.matmul(
                                pv[:, ls * 128:(ls + 1) * 128],
                                xpack[:, lc * 8192 + i * 512 + ls * 128:
                                      lc * 8192 + i * 512 + (ls + 1) * 128],
                                wkv[:, wv_off + i * 128:wv_off + (i + 1) * 128],
                                start=(i == 0), stop=(i == NDT - 1))
                    nc.vector.tensor_copy(vpack[:, lc * 512:(lc + 1) * 512], pv)

                for lc in range(NLC):
                    if lc == 2:
                        for h in range(G):
                            g = b0_head_gen(h)
                            pending.extend([lambda g=g: _step(g)] * 3)
                    if lc == 0:     # K/V first: wq is still streaming in
                        chains = [lambda: k_chain(0), lambda: v_chain(0)] + [
                            (lambda ot=ot: q_chain(0, ot)) for ot in range(G)]
                    else:
                        chains = [(lambda ot=ot: q_chain(lc, ot)) for ot in range(G)] + [
                            lambda lc=lc: k_chain(lc), lambda lc=lc: v_chain(lc)]
                    for ch in chains:
                        pump()
                        ch()
                assert not pending

            # ---------------- Phase B+C: attention + out-projection ----------------
            with ExitStack() as ctxB:
                psS = ctxB.enter_context(tc.tile_pool(name="psS", bufs=2, space="PSUM"))
                psO = ctxB.enter_context(tc.tile_pool(name="psO", bufs=2, space="PSUM"))
                psX = ctxB.enter_context(tc.tile_pool(name="psX", bufs=2, space="PSUM"))

                def out_proj_group(lc, ets, borrow_psS=False):
                    """Output projection for l-chunk lc, e-tiles ets (even
                    count). Evicts land in [128,1024] pair tiles; one DMA per
                    et-pair (halves the Sync descriptor cost). borrow_psS
                    alternates pw tiles into the idle psS slots (final batch
                    only, when no more S-pairs need them) for a 4-deep ring."""
                    ev2 = None
                    for n, et in enumerate(ets):
                        if borrow_psS and n % 2 == 1:
                            pw = psS.tile([128, 512], dt.float32, tag="psS", name="pw")
                        else:
                            pw = psX.tile([128, 512], dt.float32, tag="psX", name="pw")
                        if FP8_WO:
                            for g2 in range(2):
                                lhsT = wopack[:, (g2 * NDT + et) * 256:
                                              (g2 * NDT + et + 1) * 256].rearrange(
                                    "p (j m) -> p j m", j=2)
                                rhs = otp[g2].rearrange("p (j l) -> p j l", j=2)[
                                    :, :, lc * 512:(lc + 1) * 512]
                                nc.tensor.matmul(
                                    pw, lhsT, rhs, start=(g2 == 0), stop=(g2 == 1),
                                    perf_mode=mybir.MatmulPerfMode.DoubleRow)
                        else:
                            for g in range(G):
                                nc.tensor.matmul(
                                    pw, wopack[:, g * D + et * 128:g * D + (et + 1) * 128],
                                    ot_sb[g][:, lc * 512:(lc + 1) * 512],
                                    start=(g == 0), stop=(g == G - 1))
                        if n % 2 == 0:
                            ev2 = evp.tile([128, 1024], dt.bfloat16, tag="ev", name="ev")
                            et0 = et
                            nc.vector.tensor_copy(ev2[:, 0:512], pw)
                        else:
                            nc.scalar.copy(ev2[:, 512:1024], pw)
                            dst = outT[et0 * 128:(et0 + 2) * 128,
                                       lc * 512:(lc + 1) * 512].rearrange(
                                "(k p) c -> p k c", k=2)
                            nc.sync.dma_start(
                                out=dst, in_=ev2.rearrange("p (k c) -> p k c", k=2))

                # each head's rowsum/recip/normalize is deferred until after
                # the NEXT head's first S-pair: the rowsum matmul waits on the
                # DVE acc chain, and emitting it too early stalls the in-order
                # PE queue (~1us per head boundary). acc/psO/psX rings
                # (bufs=2) tolerate the one-head deferral.
                fin_prev = None
                for c in range(1, NLC):
                    njt = 4 * (c + 1)
                    for h in range(G):
                        qs = qt_sb[h][:, c * 512:(c + 1) * 512]
                        po = psO.tile([128, 512], dt.float32, tag="psO", name="po")
                        acc = accp.tile([128, 512], dt.bfloat16, tag="acc", name="acc")
                        for bi in range(njt // 2):
                            pair = emit_pair(c, bi, qs, psS)
                            if bi == 1 and fin_prev is not None:
                                fin_prev()
                                fin_prev = None
                            emit_av(c, bi, njt, pair, po, acc)
                        # interleave prev chunk's out-projection: ready MMs
                        # for the PE while this head's DVE acc chain drains
                        out_proj_group(c - 1, range(4 * h, 4 * h + 4))
                        fin_prev = (lambda c=c, h=h, po=po, acc=acc:
                                    emit_fin(c, h, po, acc, psX))
                    if c == NLC - 1:
                        fin_prev()
                        fin_prev = None
                        out_proj_group(c, range(NDT), borrow_psS=True)
    _split_multi_waits(nc)
    return nc


_PROG = None


def _rope_tables():
    inv_freq = 1.0 / (THETA ** (np.arange(0, HD, 2, dtype=np.float32) / HD))
    t = np.arange(L, dtype=np.float32)
    freqs = np.outer(t, inv_freq)
    emb = np.concatenate([freqs, freqs], axis=-1)      # [L, HD]
    cos = np.cos(emb).T.copy()                         # [HD, L]
    sin = np.sin(emb).T.copy()
    sin_eff = sin.copy()
    sin_eff[:64] = -sin_eff[:64]                       # dest-indexed rotate_half sign
    return cos, sin_eff


def _tiles_to_cols(a, tp):
    """[tp*128, C] -> [128, tp*C]: row-tile i becomes column block i."""
    tpn = a.shape[0] // 128
    return np.ascontiguousarray(
        a.reshape(tpn, 128, a.shape[1]).transpose(1, 0, 2).reshape(128, -1))


def _prepare_in_maps(x, Wq, Wk, Wv, Wo):
    cos, sin_eff = _rope_tables()
    bfc = lambda a: np.ascontiguousarray(a).astype(BF16)
    x, Wq, Wk, Wv, Wo = (np.asarray(a, dtype=np.float32) for a in (x, Wq, Wk, Wv, Wo))

    tri = np.tril(np.ones((128, 128), dtype=np.float32)).T  # 1 where pj <= fq

    xb = []
    for b in range(B):
        xT = x[b].T                                     # [D, L]
        chunks = [_tiles_to_cols(xT[:, lc * 512:(lc + 1) * 512], 16) for lc in range(NLC)]
        xb.append(np.concatenate(chunks, axis=1))       # [128, 32768]

    in_maps = []
    for c in range(8):
        b, g = c // 4, c % 4
        wqT = (Wq[g * GD:(g + 1) * GD, :] * SCALE).T    # [D, GD] scale folded
        wkT = Wk[g * HD:(g + 1) * HD, :].T              # [D, HD]
        wvT = Wv[g * HD:(g + 1) * HD, :].T
        woT = Wo[:, g * GD:(g + 1) * GD].T              # [GD, D]
        if FP8_WO:
            # DoubleRow layout: col = (g2*16+et)*256 + j*128 + m holds
            # woT[g2*256 + j*128 + p, et*128 + m]; fp8 bytes ride as bf16
            wo8 = np.ascontiguousarray(
                woT.reshape(2, 2, 128, NDT, 128).transpose(2, 0, 3, 1, 4)
                .reshape(128, G * D)).astype(FP8)
            wo_cols = np.concatenate([
                wo8.view(np.uint8).view(BF16),          # [128, 4096]
                np.zeros((128, G * D // 2), dtype=BF16)], axis=1)
        else:
            wo_cols = _tiles_to_cols(woT, 4)
        pack = np.concatenate([
            bfc(_tiles_to_cols(wkT, 16)),               # 2048
            bfc(_tiles_to_cols(wvT, 16)),               # 2048
            bfc(xb[b][:, 0:8192]),                      # x chunk 0
            bfc(cos), bfc(sin_eff), bfc(tri),           # 2048+2048+128
            bfc(_tiles_to_cols(wqT, 16)),               # 8192
            bfc(xb[b][:, 8192:]),                       # x chunks 1-3
            wo_cols.astype(BF16) if wo_cols.dtype != BF16 else wo_cols,
        ], axis=1)
        assert pack.shape == (128, NCOLS), pack.shape
        in_maps.append({"inpack": pack})
    return in_maps


def _run(in_maps, **kwargs):
    global _PROG
    if _PROG is None:
        _PROG = _build_program()
    return run_bass_kernel_spmd(_PROG, in_maps, list(range(8)), **kwargs)


def _gather(res):
    out = np.zeros((B, L, D), dtype=np.float32)
    for c in range(8):
        b = c // 4
        out[b] += res.results[c]["outT"].astype(np.float32).T
    return out


def kernel(x, Wq, Wk, Wv, Wo):
    return _gather(_run(_prepare_in_maps(x, Wq, Wk, Wv, Wo)))


# revision 38
# speedup vs baseline: 1.0619x; 1.0619x over previous
"""GQA (B=2, L=2048, D=2048, H=16, KVH=4, HD=128) on 8 Trainium2 NeuronCores.

Sharding: core c = (batch b = c//4, kv-group g = c%4). Each core computes its
group's 4 query heads + 1 KV head end-to-end and a partial output projection
(Wo in-dim slice); the host sums the 4 partials per batch (tensor-parallel
unshard) -- no on-device collectives.

Structure (236us vs 348us baseline; phase A 98% / phase B+C 95% PE busy):
  - All inputs host-packed into ONE contiguous [128, 57472] bf16 DRAM tensor
    per core, ordered by first use (small wk/wv first so K/V projections run
    while the big wq block streams); ~9 fat DMA descriptors replace 185
    small ones, collapsing the 56us DMA-only prologue.
  - Rope tables shared between Q and K (attention scale folded into Wq on
    host); rope eviction split ScalarE (partition-swap copies) / VectorE
    (mults+add, cos-mult reads PSUM directly).
  - V projection accumulates 4 l-blocks into one [128,512] PSUM tile with a
    single eviction into the packed V buffer (already in [l, hd] layout).
  - Phase B is chunk-outer/head-inner. Row-sums come off the PE: VectorE
    accumulates P.T tiles in SBUF, one ones-matmul per (head,chunk) does the
    partition reduction (saves ~31us of PE streaming).
  - Causal subrange AV matmuls (no masked-region streaming, no gpsimd
    memsets; pt garbage regions are simply never read). One exp per S-pair
    (masked gaps exp'd but never read).
  - Chunk-0 attention (ACT-heavy, nothing to overlap with) is interleaved
    piecewise between phase-A projection chains on dedicated PSUM pools.
  - Output projection for chunk c-1 is interleaved between phase-B head
    iterations of chunk c, emitted BEFORE each rowsum matmul so the in-order
    PE queue has ready work while the DVE acc chain catches up.
  - bf16 output (halves the output DMA; host gathers in fp32).
Tried and rejected: fp8e4 DoubleRow out-projection (4e-2 rel err - over the
2e-2 gate - and slower: 256-col LDWEIGHTS doesn't pipeline under DR MMs).
"""

import re
from contextlib import ExitStack

import ml_dtypes
import numpy as np

import concourse.bass as bass
import concourse.tile as tile
from concourse import mybir
from concourse.bass_utils import run_bass_kernel_spmd
from bass_rust import ScopedClock, VectorClock

dt = mybir.dt
BF16 = ml_dtypes.bfloat16
FP8 = ml_dtypes.float8_e4m3   # TRN float8e4 (IEEE e4m3, max 240)

# fp8e4 DoubleRow output projection: measured 4.0e-2 rel err (over the 2e-2
# gate; DVE fp32->fp8 cast noise is ~2x the round-to-nearest estimate) AND
# slower (285us vs 245us: DR matmuls don't pipeline with their 256-col
# LDWEIGHTS here). Keep False.
FP8_WO = False

B, L, D = 2, 2048, 2048
H, KVH, HD = 16, 4, 128
G = H // KVH          # 4 query heads per kv head (= per core)
GD = G * HD           # 512: per-core q-head feature dim
THETA = 10000.0
SCALE = HD ** -0.5
NLT = L // 128        # 16 l-tiles
NDT = D // 128        # 16 d-tiles
NLC = L // 512        # 4 l-chunks

# packed input column offsets (bf16 columns of the [128, NCOLS] input),
# ordered by first use: the small K/V weights land first so the K/V
# projections run while the big wq block is still streaming in
OWK = 0                   # 16 tiles x [128, 128]
OWV = OWK + NDT * 128     # 16 tiles x [128, 128]
OX0 = OWV + NDT * 128     # x chunk 0: 16 tiles x [128, 512]
OCOS = OX0 + NDT * 512    # [128, 2048]
OSIN = OCOS + L           # [128, 2048]
OTRI = OSIN + L           # [128, 128]
OWQ = OTRI + 128          # 16 tiles x [128, 512]  (wq, scale folded in)
OX123 = OWQ + NDT * 512   # x chunks 1-3: 3 x 16 tiles x [128, 512]
OWO = OX123 + 3 * NDT * 512  # 4 tiles x [128, 2048]
NCOLS = OWO + G * D


def _patch_tile_drain():
    """walrus in this container rejects multi-wait instructions on the SP
    queue; split the TileContext exit drain into one drain per proc."""
    def _drain_and_barrier_split(self, tick_clock, wait_clock):
        ticks = [int(s) for s in re.findall(r"\d+", str(tick_clock.global_clock))]
        for proc, t in enumerate(ticks):
            if t <= 0:
                continue
            vc = VectorClock()
            vc.require_at_least(proc, t)
            d = self.nc.sync.drain()
            wait_clock.add_sem_waits(d.ins, ScopedClock({None: vc}))
        self.nc.all_engine_barrier()
        assert self.sems is not None
        popped = self.nc._tile_sem_poison_stack.pop()
        assert popped is self._sem_poison
        self.nc.clear_and_free_semaphores(list(self.sems.allocated().values()))
        self.nc.all_engine_barrier()

    tile.TileContext._drain_and_barrier = _drain_and_barrier_split


def _split_multi_waits(nc):
    """This walrus build supports one sem-wait command per instruction; hoist
    excess waits onto same-engine NoOps inserted immediately before."""
    uid = 0
    for fn in nc.m.functions:
        for bb in fn.blocks:
            out = []
            for inst in bb.instructions:
                si = inst.sync_info
                if si is not None and si.on_wait and len(si.on_wait) > 1:
                    for w in si.on_wait[:-1]:
                        nop = mybir.InstNoOp(name=f"waitsplit-{uid}", ins=[], outs=[])
                        uid += 1
                        nop.engine = inst.engine
                        nop.sync_info = mybir.SyncInfo(on_wait=[w], on_update=[])
                        out.append(nop)
                    inst.sync_info = mybir.SyncInfo(
                        on_wait=[si.on_wait[-1]], on_update=si.on_update)
                out.append(inst)
            bb.instructions[:] = out


def _build_program():
    _patch_tile_drain()
    nc = bass.Bass("TRN2", target_bir_lowering=False, debug=False)

    inpack = nc.dram_tensor("inpack", [128, NCOLS], dt.bfloat16, kind="ExternalInput").ap()
    outT = nc.dram_tensor("outT", [D, L], dt.bfloat16, kind="ExternalOutput").ap()

    with tile.TileContext(nc) as tc:
        with ExitStack() as ctx:
            persist = ctx.enter_context(tc.tile_pool(name="persist", bufs=1))

            # --- persistent SBUF residents ---
            wkv = persist.tile([128, 2 * NDT * 128], dt.bfloat16, tag="wkv", name="wkv")
            trig = persist.tile([128, 2 * L + 128], dt.bfloat16, tag="trig", name="trig")
            wqpack = persist.tile([128, NDT * 512], dt.bfloat16, tag="wqpack", name="wqpack")
            xpack = persist.tile([128, NLC * NDT * 512], dt.bfloat16, tag="xpack", name="xpack")
            wo_dt = dt.float8e4 if FP8_WO else dt.bfloat16
            wo_cols = G * D // 2 if FP8_WO else G * D
            wopack = persist.tile([128, wo_cols * (2 if FP8_WO else 1)], wo_dt,
                                  tag="wopack", name="wopack")
            ones_sb = persist.tile([128, 128], dt.bfloat16, tag="ones", name="ones")
            qt_sb = [persist.tile([HD, L], dt.bfloat16, tag=f"qt{h}", name=f"qt{h}") for h in range(G)]
            kt_sb = persist.tile([HD, L], dt.bfloat16, tag="kt", name="kt")
            vpack = persist.tile([128, L], dt.bfloat16, tag="vpack", name="vpack")
            if FP8_WO:
                # two head-pair tiles: [head 2*g2 | head 2*g2+1] along columns
                otp = [persist.tile([128, 2 * L], dt.float8e4, tag=f"otp{g2}", name=f"otp{g2}")
                       for g2 in range(2)]
            else:
                ot_sb = [persist.tile([HD, L], dt.bfloat16, tag=f"ot{h}", name=f"ot{h}")
                         for h in range(G)]

            # --- input DMAs, ordered by first use (single queue runs them
            # FIFO): wk/wv -> x0 (2 halves) -> rope/tri -> wq -> x1..x3 -> wo
            nc.sync.dma_start(out=wkv, in_=inpack[:, OWK:OX0])
            for q in range(4):      # x0 in quarters: smoother chain pacing
                nc.sync.dma_start(
                    out=xpack[:, q * 2048:(q + 1) * 2048],
                    in_=inpack[:, OX0 + q * 2048:OX0 + (q + 1) * 2048])
            nc.sync.dma_start(out=trig, in_=inpack[:, OCOS:OWQ])
            nc.sync.dma_start(out=wqpack, in_=inpack[:, OWQ:OX123])
            for lc in range(1, NLC):
                nc.sync.dma_start(
                    out=xpack[:, lc * 8192:(lc + 1) * 8192],
                    in_=inpack[:, OX123 + (lc - 1) * 8192:OX123 + lc * 8192])
            if FP8_WO:
                # wo8 bytes ride in the bf16 inpack (half the columns)
                nc.sync.dma_start(
                    out=wopack, in_=inpack[:, OWO:OWO + G * D // 2].bitcast(dt.float8e4))
            else:
                nc.sync.dma_start(out=wopack, in_=inpack[:, OWO:NCOLS])
            nc.vector.memset(ones_sb, 1.0)

            cos_sb = trig[:, 0:L]
            sin_sb = trig[:, L:2 * L]
            tri_sb = trig[:, 2 * L:2 * L + 128]
            wk_off = 0            # wk tiles at wkv[:, i*128:...]
            wv_off = OWV - OWK

            def xsl(lc, i):          # moving x tile [128, 512]
                return xpack[:, lc * 8192 + i * 512: lc * 8192 + (i + 1) * 512]

            # SBUF working pools shared by phase A-embedded B0 and phase B/C
            ptp = ctx.enter_context(tc.tile_pool(name="pt", bufs=3))
            accp = ctx.enter_context(tc.tile_pool(name="acc", bufs=2))
            smp = ctx.enter_context(tc.tile_pool(name="sm", bufs=2))
            evp = ctx.enter_context(tc.tile_pool(name="ev", bufs=6))

            # ---- shared attention emit helpers (used for c=0 inside phase A
            # and c=1..3 in the main loop) ----
            def emit_pair(c, bi, qs, psS_p):
                jts = [2 * bi, 2 * bi + 1]
                offs = [(jt - 4 * c) * 128 if jt >= 4 * c else 0 for jt in jts]
                ps = psS_p.tile([128, 1024], dt.float32, tag="psS", name="ps")
                pt = ptp.tile([128, 1024], dt.bfloat16, tag="pt", name="pt")
                for k, (jt, off) in enumerate(zip(jts, offs)):
                    nc.tensor.matmul(
                        ps[:, k * 512 + off:(k + 1) * 512],
                        kt_sb[:, jt * 128:(jt + 1) * 128],
                        qs[:, off:], start=True, stop=True)
                # one exp per pair; masked gap regions get exp'd too (pairs
                # never straddle the diagonal: 4c is even) but are never read
                nc.scalar.activation(
                    pt[:, offs[0]:], ps[:, offs[0]:],
                    mybir.ActivationFunctionType.Exp)
                for k, (jt, off) in enumerate(zip(jts, offs)):
                    if jt >= 4 * c:
                        blk = pt[:, k * 512 + off:k * 512 + off + 128]
                        nc.vector.tensor_tensor(blk, blk, tri_sb, mybir.AluOpType.mult)
                return pt, jts, offs

            def emit_av(c, bi, njt, pair, po, acc):
                pt, jts, offs = pair
                last_bi = bi == njt // 2 - 1
                for k, (jt, off) in enumerate(zip(jts, offs)):
                    pk = pt[:, k * 512 + off:(k + 1) * 512]
                    nc.tensor.matmul(
                        po[:, off:], vpack[:, jt * 128:(jt + 1) * 128], pk,
                        start=(bi == 0 and k == 0), stop=(last_bi and k == 1))
                    if bi == 0 and k == 0:
                        nc.vector.tensor_copy(acc, pk)
                    else:
                        nc.vector.tensor_tensor(
                            acc[:, off:], acc[:, off:], pk, mybir.AluOpType.add)

            def emit_fin(c, h, po, acc, psX_p):
                pr = psX_p.tile([128, 512], dt.float32, tag="psX", name="pr")
                nc.tensor.matmul(pr, ones_sb, acc, start=True, stop=True)
                lnr = smp.tile([128, 512], dt.float32, tag="lnr", name="lnr")
                nc.scalar.activation(lnr, pr, mybir.ActivationFunctionType.Ln)
                rcp = smp.tile([128, 512], dt.float32, tag="rcp", name="rcp")
                nc.scalar.activation(rcp, lnr, mybir.ActivationFunctionType.Exp, scale=-1.0)
                if FP8_WO:
                    odst = otp[h // 2][:, (h % 2) * L + c * 512:(h % 2) * L + (c + 1) * 512]
                else:
                    odst = ot_sb[h][:, c * 512:(c + 1) * 512]
                nc.vector.tensor_tensor(odst, po, rcp, mybir.AluOpType.mult)

            # ---------------- Phase A: projections + rope, with chunk-0
            # attention interleaved piecewise between projection chains ----
            with ExitStack() as ctxA:
                psA = ctxA.enter_context(tc.tile_pool(name="psA", bufs=2, space="PSUM"))
                psB0s = ctxA.enter_context(tc.tile_pool(name="psB0s", bufs=1, space="PSUM"))
                psB0o = ctxA.enter_context(tc.tile_pool(name="psB0o", bufs=1, space="PSUM"))
                psB0x = ctxA.enter_context(tc.tile_pool(name="psB0x", bufs=1, space="PSUM"))
                ropep = ctxA.enter_context(tc.tile_pool(name="rope", bufs=2))

                # HAM warmup while input DMA streams: ~3.4us of dense dummy
                # matmuls flips the PE clock gate to 8/8, then a sparse
                # MM->copy->MM dependency chain keeps it warm until real work
                # arrives (real MMs start ~15us in; cold-start costs ~1.7us).
                warm = ropep.tile([64, 64], dt.bfloat16, tag="warm", name="warm")
                nc.vector.memset(warm, 0.001)
                warm_ps = psB0x.tile([64, 64], dt.float32, tag="psX", name="pwm")
                for i in range(60):
                    nc.tensor.matmul(warm_ps, warm, warm,
                                     start=(i == 0), stop=(i == 59))
                for r in range(14):
                    wc = ropep.tile([64, 64], dt.bfloat16, tag="warm", name="wc")
                    nc.vector.tensor_copy(wc, warm_ps)
                    warm_ps = psB0x.tile([64, 64], dt.float32, tag="psX", name="pwm")
                    nc.tensor.matmul(warm_ps, wc, wc, start=True, stop=True)

                def rope_evict(ps, dst_slice, lc):
                    cs = cos_sb[:, lc * 512:(lc + 1) * 512]
                    sn = sin_sb[:, lc * 512:(lc + 1) * 512]
                    swp = ropep.tile([128, 512], dt.bfloat16, tag="swp", name="swp")
                    nc.scalar.copy(swp[0:64, :], ps[64:128, :])
                    nc.scalar.copy(swp[64:128, :], ps[0:64, :])
                    t1 = ropep.tile([128, 512], dt.bfloat16, tag="t1", name="t1")
                    t2 = ropep.tile([128, 512], dt.bfloat16, tag="t2", name="t2")
                    nc.vector.tensor_tensor(t1, swp, sn, mybir.AluOpType.mult)
                    nc.vector.tensor_tensor(t2, ps, cs, mybir.AluOpType.mult)
                    nc.vector.tensor_tensor(dst_slice, t1, t2, mybir.AluOpType.add)

                def b0_head_gen(h):
                    """Chunk-0 attention for head h in 3 pieces (yield = piece
                    boundary); runs on A-phase PSUM pools."""
                    qs = qt_sb[h][:, 0:512]
                    po = psB0o.tile([128, 512], dt.float32, tag="psO", name="po")
                    acc = accp.tile([128, 512], dt.bfloat16, tag="acc", name="acc")
                    pairs = []
                    for bi in range(2):
                        pairs.append(emit_pair(0, bi, qs, psB0s))
                        yield
                        emit_av(0, bi, 4, pairs[bi], po, acc)
                    emit_fin(0, h, po, acc, psB0x)

                def _step(g):
                    try:
                        next(g)
                    except StopIteration:
                        pass

                pending = []

                def pump():
                    if pending:
                        pending.pop(0)()

                def q_chain(lc, ot):
                    ps = psA.tile([128, 512], dt.float32, tag="psA", name="psA")
                    for i in range(NDT):
                        nc.tensor.matmul(
                            ps, wqpack[:, i * 512 + ot * 128:i * 512 + (ot + 1) * 128],
                            xsl(lc, i), start=(i == 0), stop=(i == NDT - 1))
                    rope_evict(ps, qt_sb[ot][:, lc * 512:(lc + 1) * 512], lc)

                def k_chain(lc):
                    ps = psA.tile([128, 512], dt.float32, tag="psA", name="psA")
                    for i in range(NDT):
                        nc.tensor.matmul(
                            ps, wkv[:, wk_off + i * 128:wk_off + (i + 1) * 128],
                            xsl(lc, i), start=(i == 0), stop=(i == NDT - 1))
                    rope_evict(ps, kt_sb[:, lc * 512:(lc + 1) * 512], lc)

                def v_chain(lc):
                    pv = psA.tile([128, 512], dt.float32, tag="psV", bufs=2, name="psV")
                    for ls in range(4):
                        for i in range(NDT):
                            nc.tensor.matmul(
                                pv[:, ls * 128:(ls + 1) * 128],
                                xpack[:, lc * 8192 + i * 512 + ls * 128:
                                      lc * 8192 + i * 512 + (ls + 1) * 128],
                                wkv[:, wv_off + i * 128:wv_off + (i + 1) * 128],
                                start=(i == 0), stop=(i == NDT - 1))
                    nc.vector.tensor_copy(vpack[:, lc * 512:(lc + 1) * 512], pv)

                for lc in range(NLC):
                    if lc == 2:
                        for h in range(G):
                            g = b0_head_gen(h)
                            pending.extend([lambda g=g: _step(g)] * 3)
                    if lc == 0:     # K/V first: wq is still streaming in
                        chains = [lambda: k_chain(0), lambda: v_chain(0)] + [
                            (lambda ot=ot: q_chain(0, ot)) for ot in range(G)]
                    else:
                        chains = [(lambda ot=ot: q_chain(lc, ot)) for ot in range(G)] + [
                            lambda lc=lc: k_chain(lc), lambda lc=lc: v_chain(lc)]
                    for ch in chains:
                        pump()
                        ch()
                assert not pending

            # ---------------- Phase B+C: attention + out-projection ----------------
            with ExitStack() as ctxB:
                psS = ctxB.enter_context(tc.tile_pool(name="psS", bufs=2, space="PSUM"))
                psO = ctxB.enter_context(tc.tile_pool(name="psO", bufs=2, space="PSUM"))
                psX = ctxB.enter_context(tc.tile_pool(name="psX", bufs=2, space="PSUM"))

                def out_proj_group(lc, ets, borrow_psS=False):
                    """Output projection for l-chunk lc, e-tiles ets (even
                    count). Evicts land in [128,1024] pair tiles; one DMA per
                    et-pair (halves the Sync descriptor cost). borrow_psS
                    alternates pw tiles into the idle psS slots (final batch
                    only, when no more S-pairs need them) for a 4-deep ring."""
                    ev2 = None
                    for n, et in enumerate(ets):
                        if borrow_psS and n % 2 == 1:
                            pw = psS.tile([128, 512], dt.float32, tag="psS", name="pw")
                        else:
                            pw = psX.tile([128, 512], dt.float32, tag="psX", name="pw")
                        if FP8_WO:
                            for g2 in range(2):
                                lhsT = wopack[:, (g2 * NDT + et) * 256:
                                              (g2 * NDT + et + 1) * 256].rearrange(
                                    "p (j m) -> p j m", j=2)
                                rhs = otp[g2].rearrange("p (j l) -> p j l", j=2)[
                                    :, :, lc * 512:(lc + 1) * 512]
                                nc.tensor.matmul(
                                    pw, lhsT, rhs, start=(g2 == 0), stop=(g2 == 1),
                                    perf_mode=mybir.MatmulPerfMode.DoubleRow)
                        else:
                            for g in range(G):
                                nc.tensor.matmul(
                                    pw, wopack[:, g * D + et * 128:g * D + (et + 1) * 128],
                                    ot_sb[g][:, lc * 512:(lc + 1) * 512],
                                    start=(g == 0), stop=(g == G - 1))
                        if n % 2 == 0:
                            ev2 = evp.tile([128, 1024], dt.bfloat16, tag="ev", name="ev")
                            et0 = et
                            nc.vector.tensor_copy(ev2[:, 0:512], pw)
                        else:
                            nc.scalar.copy(ev2[:, 512:1024], pw)
                            dst = outT[et0 * 128:(et0 + 2) * 128,
                                       lc * 512:(lc + 1) * 512].rearrange(
                                "(k p) c -> p k c", k=2)
                            nc.sync.dma_start(
                                out=dst, in_=ev2.rearrange("p (k c) -> p k c", k=2))

                for c in range(1, NLC):
                    njt = 4 * (c + 1)
                    for h in range(G):
                        qs = qt_sb[h][:, c * 512:(c + 1) * 512]
                        po = psO.tile([128, 512], dt.float32, tag="psO", name="po")
                        acc = accp.tile([128, 512], dt.bfloat16, tag="acc", name="acc")
                        for bi in range(njt // 2):
                            pair = emit_pair(c, bi, qs, psS)
                            emit_av(c, bi, njt, pair, po, acc)
                        # interleave prev chunk's out-projection BEFORE the
                        # rowsum matmul: the in-order PE queue chews these
                        # ready MMs while the DVE acc chain catches up
                        out_proj_group(c - 1, range(4 * h, 4 * h + 4))
                        emit_fin(c, h, po, acc, psX)
                    if c == NLC - 1:    # last chunk's out-projection runs at the end
                        out_proj_group(c, range(NDT), borrow_psS=True)
    _split_multi_waits(nc)
    return nc


_PROG = None


def _rope_tables():
    inv_freq = 1.0 / (THETA ** (np.arange(0, HD, 2, dtype=np.float32) / HD))
    t = np.arange(L, dtype=np.float32)
    freqs = np.outer(t, inv_freq)
    emb = np.concatenate([freqs, freqs], axis=-1)      # [L, HD]
    cos = np.cos(emb).T.copy()                         # [HD, L]
    sin = np.sin(emb).T.copy()
    sin_eff = sin.copy()
    sin_eff[:64] = -sin_eff[:64]                       # dest-indexed rotate_half sign
    return cos, sin_eff


def _tiles_to_cols(a, tp):
    """[tp*128, C] -> [128, tp*C]: row-tile i becomes column block i."""
    tpn = a.shape[0] // 128
    return np.ascontiguousarray(
        a.reshape(tpn, 128, a.shape[1]).transpose(1, 0, 2).reshape(128, -1))


def _prepare_in_maps(x, Wq, Wk, Wv, Wo):
    cos, sin_eff = _rope_tables()
    bfc = lambda a: np.ascontiguousarray(a).astype(BF16)
    x, Wq, Wk, Wv, Wo = (np.asarray(a, dtype=np.float32) for a in (x, Wq, Wk, Wv, Wo))

    tri = np.tril(np.ones((128, 128), dtype=np.float32)).T  # 1 where pj <= fq

    xb = []
    for b in range(B):
        xT = x[b].T                                     # [D, L]
        chunks = [_tiles_to_cols(xT[:, lc * 512:(lc + 1) * 512], 16) for lc in range(NLC)]
        xb.append(np.concatenate(chunks, axis=1))       # [128, 32768]

    in_maps = []
    for c in range(8):
        b, g = c // 4, c % 4
        wqT = (Wq[g * GD:(g + 1) * GD, :] * SCALE).T    # [D, GD] scale folded
        wkT = Wk[g * HD:(g + 1) * HD, :].T              # [D, HD]
        wvT = Wv[g * HD:(g + 1) * HD, :].T
        woT = Wo[:, g * GD:(g + 1) * GD].T              # [GD, D]
        if FP8_WO:
            # DoubleRow layout: col = (g2*16+et)*256 + j*128 + m holds
            # woT[g2*256 + j*128 + p, et*128 + m]; fp8 bytes ride as bf16
            wo8 = np.ascontiguousarray(
                woT.reshape(2, 2, 128, NDT, 128).transpose(2, 0, 3, 1, 4)
                .reshape(128, G * D)).astype(FP8)
            wo_cols = np.concatenate([
                wo8.view(np.uint8).view(BF16),          # [128, 4096]
                np.zeros((128, G * D // 2), dtype=BF16)], axis=1)
        else:
            wo_cols = _tiles_to_cols(woT, 4)
        pack = np.concatenate([
            bfc(_tiles_to_cols(wkT, 16)),               # 2048
            bfc(_tiles_to_cols(wvT, 16)),               # 2048
            bfc(xb[b][:, 0:8192]),                      # x chunk 0
            bfc(cos), bfc(sin_eff), bfc(tri),           # 2048+2048+128
            bfc(_tiles_to_cols(wqT, 16)),               # 8192
            bfc(xb[b][:, 8192:]),                       # x chunks 1-3
            wo_cols.astype(BF16) if wo_cols.dtype != BF16 else wo_cols,
        ], axis=1)
        assert pack.shape == (128, NCOLS), pack.shape
        in_maps.append({"inpack": pack})
    return in_maps


def _run(in_maps, **kwargs):
    global _PROG
    if _PROG is None:
        _PROG = _build_program()
    return run_bass_kernel_spmd(_PROG, in_maps, list(range(8)), **kwargs)


def _gather(res):
    out = np.zeros((B, L, D), dtype=np.float32)
    for c in range(8):
        b = c // 4
        out[b] += res.results[c]["outT"].astype(np.float32).T
    return out


def kernel(x, Wq, Wk, Wv, Wo):
    return _gather(_run(_prepare_in_maps(x, Wq, Wk, Wv, Wo)))


# revision 43
# speedup vs baseline: 1.0762x; 1.0135x over previous
"""GQA (B=2, L=2048, D=2048, H=16, KVH=4, HD=128) on 8 Trainium2 NeuronCores.

Sharding: core c = (batch b = c//4, kv-group g = c%4). Each core computes its
group's 4 query heads + 1 KV head end-to-end and a partial output projection
(Wo in-dim slice); the host sums the 4 partials per batch (tensor-parallel
unshard) -- no on-device collectives.

Structure (236us vs 348us baseline; phase A 98% / phase B+C 95% PE busy):
  - All inputs host-packed into ONE contiguous [128, 57472] bf16 DRAM tensor
    per core, ordered by first use (small wk/wv first so K/V projections run
    while the big wq block streams); ~9 fat DMA descriptors replace 185
    small ones, collapsing the 56us DMA-only prologue.
  - Rope tables shared between Q and K (attention scale folded into Wq on
    host); rope eviction split ScalarE (partition-swap copies) / VectorE
    (mults+add, cos-mult reads PSUM directly).
  - V projection accumulates 4 l-blocks into one [128,512] PSUM tile with a
    single eviction into the packed V buffer (already in [l, hd] layout).
  - Phase B is chunk-outer/head-inner. Row-sums come off the PE: VectorE
    accumulates P.T tiles in SBUF, one ones-matmul per (head,chunk) does the
    partition reduction (saves ~31us of PE streaming).
  - Causal subrange AV matmuls (no masked-region streaming, no gpsimd
    memsets; pt garbage regions are simply never read). One exp per S-pair
    (masked gaps exp'd but never read).
  - Chunk-0 attention (ACT-heavy, nothing to overlap with) is interleaved
    piecewise between phase-A projection chains on dedicated PSUM pools.
  - Output projection for chunk c-1 is interleaved between phase-B head
    iterations of chunk c, emitted BEFORE each rowsum matmul so the in-order
    PE queue has ready work while the DVE acc chain catches up.
  - bf16 output (halves the output DMA; host gathers in fp32).
Tried and rejected: fp8e4 DoubleRow out-projection (4e-2 rel err - over the
2e-2 gate - and slower: 256-col LDWEIGHTS doesn't pipeline under DR MMs).
"""

import re
from contextlib import ExitStack

import ml_dtypes
import numpy as np

import concourse.bass as bass
import concourse.tile as tile
from concourse import mybir
from concourse.bass_utils import run_bass_kernel_spmd
from bass_rust import ScopedClock, VectorClock

dt = mybir.dt
BF16 = ml_dtypes.bfloat16
FP8 = ml_dtypes.float8_e4m3   # TRN float8e4 (IEEE e4m3, max 240)

# fp8e4 DoubleRow output projection: measured 4.0e-2 rel err (over the 2e-2
# gate; DVE fp32->fp8 cast noise is ~2x the round-to-nearest estimate) AND
# slower (285us vs 245us: DR matmuls don't pipeline with their 256-col
# LDWEIGHTS here). Keep False.
FP8_WO = False

B, L, D = 2, 2048, 2048
H, KVH, HD = 16, 4, 128
G = H // KVH          # 4 query heads per kv head (= per core)
GD = G * HD           # 512: per-core q-head feature dim
THETA = 10000.0
SCALE = HD ** -0.5
NLT = L // 128        # 16 l-tiles
NDT = D // 128        # 16 d-tiles
NLC = L // 512        # 4 l-chunks

# packed input column offsets (bf16 columns of the [128, NCOLS] input),
# ordered by first use: the small K/V weights land first so the K/V
# projections run while the big wq block is still streaming in
OWK = 0                   # 16 tiles x [128, 128]
OWV = OWK + NDT * 128     # 16 tiles x [128, 128]
OX0 = OWV + NDT * 128     # x chunk 0: 16 tiles x [128, 512]
OWQ = OX0 + NDT * 512     # 16 tiles x [128, 512]  (wq, scale folded in)
OCOS = OWQ + NDT * 512    # [128, 2048]  (rope tables after wq: first
OSIN = OCOS + L           # consumer is the K eviction, not a matmul chain,
OTRI = OSIN + L           # and phase A's end is pinned by wq arrival)
OX123 = OTRI + 128        # x chunks 1-3: 3 x 16 tiles x [128, 512]
OWO = OX123 + 3 * NDT * 512  # 4 tiles x [128, 2048]
NCOLS = OWO + G * D


def _patch_tile_drain():
    """walrus in this container rejects multi-wait instructions on the SP
    queue; split the TileContext exit drain into one drain per proc."""
    def _drain_and_barrier_split(self, tick_clock, wait_clock):
        ticks = [int(s) for s in re.findall(r"\d+", str(tick_clock.global_clock))]
        for proc, t in enumerate(ticks):
            if t <= 0:
                continue
            vc = VectorClock()
            vc.require_at_least(proc, t)
            d = self.nc.sync.drain()
            wait_clock.add_sem_waits(d.ins, ScopedClock({None: vc}))
        self.nc.all_engine_barrier()
        assert self.sems is not None
        popped = self.nc._tile_sem_poison_stack.pop()
        assert popped is self._sem_poison
        self.nc.clear_and_free_semaphores(list(self.sems.allocated().values()))
        self.nc.all_engine_barrier()

    tile.TileContext._drain_and_barrier = _drain_and_barrier_split


def _split_multi_waits(nc):
    """This walrus build supports one sem-wait command per instruction; hoist
    excess waits onto same-engine NoOps inserted immediately before."""
    uid = 0
    for fn in nc.m.functions:
        for bb in fn.blocks:
            out = []
            for inst in bb.instructions:
                si = inst.sync_info
                if si is not None and si.on_wait and len(si.on_wait) > 1:
                    for w in si.on_wait[:-1]:
                        nop = mybir.InstNoOp(name=f"waitsplit-{uid}", ins=[], outs=[])
                        uid += 1
                        nop.engine = inst.engine
                        nop.sync_info = mybir.SyncInfo(on_wait=[w], on_update=[])
                        out.append(nop)
                    inst.sync_info = mybir.SyncInfo(
                        on_wait=[si.on_wait[-1]], on_update=si.on_update)
                out.append(inst)
            bb.instructions[:] = out


def _build_program():
    _patch_tile_drain()
    nc = bass.Bass("TRN2", target_bir_lowering=False, debug=False)

    inpack = nc.dram_tensor("inpack", [128, NCOLS], dt.bfloat16, kind="ExternalInput").ap()
    outT = nc.dram_tensor("outT", [D, L], dt.bfloat16, kind="ExternalOutput").ap()

    with tile.TileContext(nc) as tc:
        with ExitStack() as ctx:
            persist = ctx.enter_context(tc.tile_pool(name="persist", bufs=1))

            # --- persistent SBUF residents ---
            wkv = persist.tile([128, 2 * NDT * 128], dt.bfloat16, tag="wkv", name="wkv")
            trig = persist.tile([128, 2 * L + 128], dt.bfloat16, tag="trig", name="trig")
            wqpack = persist.tile([128, NDT * 512], dt.bfloat16, tag="wqpack", name="wqpack")
            xpack = persist.tile([128, NLC * NDT * 512], dt.bfloat16, tag="xpack", name="xpack")
            wo_dt = dt.float8e4 if FP8_WO else dt.bfloat16
            wo_cols = G * D // 2 if FP8_WO else G * D
            wopack = persist.tile([128, wo_cols * (2 if FP8_WO else 1)], wo_dt,
                                  tag="wopack", name="wopack")
            ones_sb = persist.tile([128, 128], dt.bfloat16, tag="ones", name="ones")
            qt_sb = [persist.tile([HD, L], dt.bfloat16, tag=f"qt{h}", name=f"qt{h}") for h in range(G)]
            kt_sb = persist.tile([HD, L], dt.bfloat16, tag="kt", name="kt")
            vpack = persist.tile([128, L], dt.bfloat16, tag="vpack", name="vpack")
            if FP8_WO:
                # two head-pair tiles: [head 2*g2 | head 2*g2+1] along columns
                otp = [persist.tile([128, 2 * L], dt.float8e4, tag=f"otp{g2}", name=f"otp{g2}")
                       for g2 in range(2)]
            else:
                ot_sb = [persist.tile([HD, L], dt.bfloat16, tag=f"ot{h}", name=f"ot{h}")
                         for h in range(G)]

            # --- input DMAs, ordered by first use (single queue runs them
            # FIFO): wk/wv -> x0 (2 halves) -> rope/tri -> wq -> x1..x3 -> wo
            nc.sync.dma_start(out=wkv, in_=inpack[:, OWK:OX0])
            for q in range(4):      # x0 in quarters: smoother chain pacing
                nc.sync.dma_start(
                    out=xpack[:, q * 2048:(q + 1) * 2048],
                    in_=inpack[:, OX0 + q * 2048:OX0 + (q + 1) * 2048])
            nc.sync.dma_start(out=wqpack, in_=inpack[:, OWQ:OCOS])
            nc.sync.dma_start(out=trig, in_=inpack[:, OCOS:OX123])
            for lc in range(1, NLC):
                nc.sync.dma_start(
                    out=xpack[:, lc * 8192:(lc + 1) * 8192],
                    in_=inpack[:, OX123 + (lc - 1) * 8192:OX123 + lc * 8192])
            if FP8_WO:
                # wo8 bytes ride in the bf16 inpack (half the columns)
                nc.sync.dma_start(
                    out=wopack, in_=inpack[:, OWO:OWO + G * D // 2].bitcast(dt.float8e4))
            else:
                nc.sync.dma_start(out=wopack, in_=inpack[:, OWO:NCOLS])
            nc.vector.memset(ones_sb, 1.0)

            cos_sb = trig[:, 0:L]
            sin_sb = trig[:, L:2 * L]
            tri_sb = trig[:, 2 * L:2 * L + 128]
            wk_off = 0            # wk tiles at wkv[:, i*128:...]
            wv_off = OWV - OWK

            def xsl(lc, i):          # moving x tile [128, 512]
                return xpack[:, lc * 8192 + i * 512: lc * 8192 + (i + 1) * 512]

            # SBUF working pools shared by phase A-embedded B0 and phase B/C
            ptp = ctx.enter_context(tc.tile_pool(name="pt", bufs=3))
            accp = ctx.enter_context(tc.tile_pool(name="acc", bufs=2))
            smp = ctx.enter_context(tc.tile_pool(name="sm", bufs=2))
            evp = ctx.enter_context(tc.tile_pool(name="ev", bufs=6))

            # ---- shared attention emit helpers (used for c=0 inside phase A
            # and c=1..3 in the main loop) ----
            def emit_pair(c, bi, qs, psS_p):
                jts = [2 * bi, 2 * bi + 1]
                offs = [(jt - 4 * c) * 128 if jt >= 4 * c else 0 for jt in jts]
                ps = psS_p.tile([128, 1024], dt.float32, tag="psS", name="ps")
                pt = ptp.tile([128, 1024], dt.bfloat16, tag="pt", name="pt")
                for k, (jt, off) in enumerate(zip(jts, offs)):
                    nc.tensor.matmul(
                        ps[:, k * 512 + off:(k + 1) * 512],
                        kt_sb[:, jt * 128:(jt + 1) * 128],
                        qs[:, off:], start=True, stop=True)
                # one exp per pair; masked gap regions get exp'd too (pairs
                # never straddle the diagonal: 4c is even) but are never read
                nc.scalar.activation(
                    pt[:, offs[0]:], ps[:, offs[0]:],
                    mybir.ActivationFunctionType.Exp)
                for k, (jt, off) in enumerate(zip(jts, offs)):
                    if jt >= 4 * c:
                        blk = pt[:, k * 512 + off:k * 512 + off + 128]
                        nc.vector.tensor_tensor(blk, blk, tri_sb, mybir.AluOpType.mult)
                return pt, jts, offs

            def emit_av(c, bi, njt, pair, po, acc):
                pt, jts, offs = pair
                last_bi = bi == njt // 2 - 1
                for k, (jt, off) in enumerate(zip(jts, offs)):
                    pk = pt[:, k * 512 + off:(k + 1) * 512]
                    nc.tensor.matmul(
                        po[:, off:], vpack[:, jt * 128:(jt + 1) * 128], pk,
                        start=(bi == 0 and k == 0), stop=(last_bi and k == 1))
                    if bi == 0 and k == 0:
                        nc.vector.tensor_copy(acc, pk)
                    else:
                        nc.vector.tensor_tensor(
                            acc[:, off:], acc[:, off:], pk, mybir.AluOpType.add)

            def emit_fin(c, h, po, acc, psX_p):
                pr = psX_p.tile([128, 512], dt.float32, tag="psX", name="pr")
                nc.tensor.matmul(pr, ones_sb, acc, start=True, stop=True)
                lnr = smp.tile([128, 512], dt.float32, tag="lnr", name="lnr")
                nc.scalar.activation(lnr, pr, mybir.ActivationFunctionType.Ln)
                rcp = smp.tile([128, 512], dt.float32, tag="rcp", name="rcp")
                nc.scalar.activation(rcp, lnr, mybir.ActivationFunctionType.Exp, scale=-1.0)
                if FP8_WO:
                    odst = otp[h // 2][:, (h % 2) * L + c * 512:(h % 2) * L + (c + 1) * 512]
                else:
                    odst = ot_sb[h][:, c * 512:(c + 1) * 512]
                nc.vector.tensor_tensor(odst, po, rcp, mybir.AluOpType.mult)

            # ---------------- Phase A: projections + rope, with chunk-0
            # attention interleaved piecewise between projection chains ----
            with ExitStack() as ctxA:
                # psA bufs=3 so the K eviction (which waits on the late-
                # arriving rope tables) can hold its PSUM slot without
                # stalling the following Q chains; psV drops to 1 buf (V
                # evictions have ~22us of slack between chunks)
                psA = ctxA.enter_context(tc.tile_pool(name="psA", bufs=3, space="PSUM"))
                psB0s = ctxA.enter_context(tc.tile_pool(name="psB0s", bufs=1, space="PSUM"))
                psB0o = ctxA.enter_context(tc.tile_pool(name="psB0o", bufs=1, space="PSUM"))
                psB0x = ctxA.enter_context(tc.tile_pool(name="psB0x", bufs=1, space="PSUM"))
                ropep = ctxA.enter_context(tc.tile_pool(name="rope", bufs=2))

                def rope_evict(ps, dst_slice, lc):
                    cs = cos_sb[:, lc * 512:(lc + 1) * 512]
                    sn = sin_sb[:, lc * 512:(lc + 1) * 512]
                    swp = ropep.tile([128, 512], dt.bfloat16, tag="swp", name="swp")
                    nc.scalar.copy(swp[0:64, :], ps[64:128, :])
                    nc.scalar.copy(swp[64:128, :], ps[0:64, :])
                    t1 = ropep.tile([128, 512], dt.bfloat16, tag="t1", name="t1")
                    t2 = ropep.tile([128, 512], dt.bfloat16, tag="t2", name="t2")
                    nc.vector.tensor_tensor(t1, swp, sn, mybir.AluOpType.mult)
                    nc.vector.tensor_tensor(t2, ps, cs, mybir.AluOpType.mult)
                    nc.vector.tensor_tensor(dst_slice, t1, t2, mybir.AluOpType.add)

                def b0_head_gen(h):
                    """Chunk-0 attention for head h in 3 pieces (yield = piece
                    boundary); runs on A-phase PSUM pools."""
                    qs = qt_sb[h][:, 0:512]
                    po = psB0o.tile([128, 512], dt.float32, tag="psO", name="po")
                    acc = accp.tile([128, 512], dt.bfloat16, tag="acc", name="acc")
                    pairs = []
                    for bi in range(2):
                        pairs.append(emit_pair(0, bi, qs, psB0s))
                        yield
                        emit_av(0, bi, 4, pairs[bi], po, acc)
                    emit_fin(0, h, po, acc, psB0x)

                def _step(g):
                    try:
                        next(g)
                    except StopIteration:
                        pass

                pending = []

                def pump():
                    if pending:
                        pending.pop(0)()

                def q_chain(lc, ot):
                    ps = psA.tile([128, 512], dt.float32, tag="psA", name="psA")
                    for i in range(NDT):
                        nc.tensor.matmul(
                            ps, wqpack[:, i * 512 + ot * 128:i * 512 + (ot + 1) * 128],
                            xsl(lc, i), start=(i == 0), stop=(i == NDT - 1))
                    rope_evict(ps, qt_sb[ot][:, lc * 512:(lc + 1) * 512], lc)

                def k_chain(lc):
                    ps = psA.tile([128, 512], dt.float32, tag="psA", name="psA")
                    for i in range(NDT):
                        nc.tensor.matmul(
                            ps, wkv[:, wk_off + i * 128:wk_off + (i + 1) * 128],
                            xsl(lc, i), start=(i == 0), stop=(i == NDT - 1))
                    rope_evict(ps, kt_sb[:, lc * 512:(lc + 1) * 512], lc)

                def v_chain(lc):
                    pv = psA.tile([128, 512], dt.float32, tag="psV", bufs=1, name="psV")
                    for ls in range(4):
                        for i in range(NDT):
                            nc.tensor.matmul(
                                pv[:, ls * 128:(ls + 1) * 128],
                                xpack[:, lc * 8192 + i * 512 + ls * 128:
                                      lc * 8192 + i * 512 + (ls + 1) * 128],
                                wkv[:, wv_off + i * 128:wv_off + (i + 1) * 128],
                                start=(i == 0), stop=(i == NDT - 1))
                    nc.vector.tensor_copy(vpack[:, lc * 512:(lc + 1) * 512], pv)

                for lc in range(NLC):
                    if lc == 2:
                        for h in range(G):
                            g = b0_head_gen(h)
                            pending.extend([lambda g=g: _step(g)] * 3)
                    if lc == 0:     # K/V first: wq is still streaming in
                        chains = [lambda: k_chain(0), lambda: v_chain(0)] + [
                            (lambda ot=ot: q_chain(0, ot)) for ot in range(G)]
                    else:
                        chains = [(lambda ot=ot: q_chain(lc, ot)) for ot in range(G)] + [
                            lambda lc=lc: k_chain(lc), lambda lc=lc: v_chain(lc)]
                    for ch in chains:
                        pump()
                        ch()
                assert not pending

            # ---------------- Phase B+C: attention + out-projection ----------------
            with ExitStack() as ctxB:
                psS = ctxB.enter_context(tc.tile_pool(name="psS", bufs=2, space="PSUM"))
                psO = ctxB.enter_context(tc.tile_pool(name="psO", bufs=2, space="PSUM"))
                psX = ctxB.enter_context(tc.tile_pool(name="psX", bufs=2, space="PSUM"))

                def out_proj_group(lc, ets, borrow_psS=False):
                    """Output projection for l-chunk lc, e-tiles ets (even
                    count). Evicts land in [128,1024] pair tiles; one DMA per
                    et-pair (halves the Sync descriptor cost). borrow_psS
                    alternates pw tiles into the idle psS slots (final batch
                    only, when no more S-pairs need them) for a 4-deep ring."""
                    ev2 = None
                    for n, et in enumerate(ets):
                        if borrow_psS and n % 2 == 1:
                            pw = psS.tile([128, 512], dt.float32, tag="psS", name="pw")
                        else:
                            pw = psX.tile([128, 512], dt.float32, tag="psX", name="pw")
                        if FP8_WO:
                            for g2 in range(2):
                                lhsT = wopack[:, (g2 * NDT + et) * 256:
                                              (g2 * NDT + et + 1) * 256].rearrange(
                                    "p (j m) -> p j m", j=2)
                                rhs = otp[g2].rearrange("p (j l) -> p j l", j=2)[
                                    :, :, lc * 512:(lc + 1) * 512]
                                nc.tensor.matmul(
                                    pw, lhsT, rhs, start=(g2 == 0), stop=(g2 == 1),
                                    perf_mode=mybir.MatmulPerfMode.DoubleRow)
                        else:
                            for g in range(G):
                                nc.tensor.matmul(
                                    pw, wopack[:, g * D + et * 128:g * D + (et + 1) * 128],
                                    ot_sb[g][:, lc * 512:(lc + 1) * 512],
                                    start=(g == 0), stop=(g == G - 1))
                        if n % 2 == 0:
                            ev2 = evp.tile([128, 1024], dt.bfloat16, tag="ev", name="ev")
                            et0 = et
                            nc.vector.tensor_copy(ev2[:, 0:512], pw)
                        else:
                            nc.scalar.copy(ev2[:, 512:1024], pw)
                            dst = outT[et0 * 128:(et0 + 2) * 128,
                                       lc * 512:(lc + 1) * 512].rearrange(
                                "(k p) c -> p k c", k=2)
                            nc.sync.dma_start(
                                out=dst, in_=ev2.rearrange("p (k c) -> p k c", k=2))

                for c in range(1, NLC):
                    njt = 4 * (c + 1)
                    for h in range(G):
                        qs = qt_sb[h][:, c * 512:(c + 1) * 512]
                        po = psO.tile([128, 512], dt.float32, tag="psO", name="po")
                        acc = accp.tile([128, 512], dt.bfloat16, tag="acc", name="acc")
                        for bi in range(njt // 2):
                            pair = emit_pair(c, bi, qs, psS)
                            emit_av(c, bi, njt, pair, po, acc)
                        # interleave prev chunk's out-projection BEFORE the
                        # rowsum matmul: the in-order PE queue chews these
                        # ready MMs while the DVE acc chain catches up
                        out_proj_group(c - 1, range(4 * h, 4 * h + 4))
                        emit_fin(c, h, po, acc, psX)
                    if c == NLC - 1:    # last chunk's out-projection runs at the end
                        out_proj_group(c, range(NDT), borrow_psS=True)
    _split_multi_waits(nc)
    return nc


_PROG = None


def _rope_tables():
    inv_freq = 1.0 / (THETA ** (np.arange(0, HD, 2, dtype=np.float32) / HD))
    t = np.arange(L, dtype=np.float32)
    freqs = np.outer(t, inv_freq)
    emb = np.concatenate([freqs, freqs], axis=-1)      # [L, HD]
    cos = np.cos(emb).T.copy()                         # [HD, L]
    sin = np.sin(emb).T.copy()
    sin_eff = sin.copy()
    sin_eff[:64] = -sin_eff[:64]                       # dest-indexed rotate_half sign
    return cos, sin_eff


def _tiles_to_cols(a, tp):
    """[tp*128, C] -> [128, tp*C]: row-tile i becomes column block i."""
    tpn = a.shape[0] // 128
    return np.ascontiguousarray(
        a.reshape(tpn, 128, a.shape[1]).transpose(1, 0, 2).reshape(128, -1))


def _prepare_in_maps(x, Wq, Wk, Wv, Wo):
    cos, sin_eff = _rope_tables()
    bfc = lambda a: np.ascontiguousarray(a).astype(BF16)
    x, Wq, Wk, Wv, Wo = (np.asarray(a, dtype=np.float32) for a in (x, Wq, Wk, Wv, Wo))

    tri = np.tril(np.ones((128, 128), dtype=np.float32)).T  # 1 where pj <= fq

    xb = []
    for b in range(B):
        xT = x[b].T                                     # [D, L]
        chunks = [_tiles_to_cols(xT[:, lc * 512:(lc + 1) * 512], 16) for lc in range(NLC)]
        xb.append(np.concatenate(chunks, axis=1))       # [128, 32768]

    in_maps = []
    for c in range(8):
        b, g = c // 4, c % 4
        wqT = (Wq[g * GD:(g + 1) * GD, :] * SCALE).T    # [D, GD] scale folded
        wkT = Wk[g * HD:(g + 1) * HD, :].T              # [D, HD]
        wvT = Wv[g * HD:(g + 1) * HD, :].T
        woT = Wo[:, g * GD:(g + 1) * GD].T              # [GD, D]
        if FP8_WO:
            # DoubleRow layout: col = (g2*16+et)*256 + j*128 + m holds
            # woT[g2*256 + j*128 + p, et*128 + m]; fp8 bytes ride as bf16
            wo8 = np.ascontiguousarray(
                woT.reshape(2, 2, 128, NDT, 128).transpose(2, 0, 3, 1, 4)
                .reshape(128, G * D)).astype(FP8)
            wo_cols = np.concatenate([
                wo8.view(np.uint8).view(BF16),          # [128, 4096]
                np.zeros((128, G * D // 2), dtype=BF16)], axis=1)
        else:
            wo_cols = _tiles_to_cols(woT, 4)
        pack = np.concatenate([
            bfc(_tiles_to_cols(wkT, 16)),               # 2048
            bfc(_tiles_to_cols(wvT, 16)),               # 2048
            bfc(xb[b][:, 0:8192]),                      # x chunk 0
            bfc(_tiles_to_cols(wqT, 16)),               # 8192
            bfc(cos), bfc(sin_eff), bfc(tri),           # 2048+2048+128
            bfc(xb[b][:, 8192:]),                       # x chunks 1-3
            wo_cols.astype(BF16) if wo_cols.dtype != BF16 else wo_cols,
        ], axis=1)
        assert pack.shape == (128, NCOLS), pack.shape
        in_maps.append({"inpack": pack})
    return in_maps


def _run(in_maps, **kwargs):
    global _PROG
    if _PROG is None:
        _PROG = _build_program()
    return run_bass_kernel_spmd(_PROG, in_maps, list(range(8)), **kwargs)


def _gather(res):
    out = np.zeros((B, L, D), dtype=np.float32)
    for c in range(8):
        b = c // 4
        out[b] += res.results[c]["outT"].astype(np.float32).T
    return out


def kernel(x, Wq, Wk, Wv, Wo):
    return _gather(_run(_prepare_in_maps(x, Wq, Wk, Wv, Wo)))


# revision 45
# speedup vs baseline: 1.0812x; 1.0046x over previous
"""GQA (B=2, L=2048, D=2048, H=16, KVH=4, HD=128) on 8 Trainium2 NeuronCores.

Sharding: core c = (batch b = c//4, kv-group g = c%4). Each core computes its
group's 4 query heads + 1 KV head end-to-end and a partial output projection
(Wo in-dim slice); the host sums the 4 partials per batch (tensor-parallel
unshard) -- no on-device collectives.

Structure (232us vs 348us baseline; phase A 98% / phase B+C 95% PE busy):
  - All inputs host-packed into ONE contiguous [128, 57472] bf16 DRAM tensor
    per core, ordered by first use (small wk/wv first so the K/V projections
    run while wq streams; rope tables after wq since phase A's end is pinned
    by wq arrival and the tables' first consumer is an eviction, not a
    matmul chain); ~11 fat DMA descriptors replace 185 small ones,
    collapsing the 56us DMA-only prologue.
  - Rope tables shared between Q and K (attention scale folded into Wq on
    host); rope eviction split ScalarE (partition-swap copies) / VectorE
    (mults+add, cos-mult reads PSUM directly).
  - V projection accumulates 4 l-blocks into one [128,512] PSUM tile with a
    single eviction into the packed V buffer (already in [l, hd] layout).
  - Phase B is chunk-outer/head-inner. Row-sums come off the PE: VectorE
    accumulates P.T tiles in SBUF, one ones-matmul per (head,chunk) does the
    partition reduction (saves ~31us of PE streaming).
  - Causal subrange AV matmuls (no masked-region streaming, no gpsimd
    memsets; pt garbage regions are simply never read). One exp per S-pair
    (masked gaps exp'd but never read).
  - Chunk-0 attention (ACT-heavy, nothing to overlap with) is interleaved
    piecewise between phase-A projection chains on dedicated PSUM pools.
  - Output projection for chunk c-1 is interleaved between phase-B head
    iterations of chunk c, emitted BEFORE each rowsum matmul so the in-order
    PE queue has ready work while the DVE acc chain catches up.
  - bf16 output (halves the output DMA; host gathers in fp32).
Tried and rejected: fp8e4 DoubleRow out-projection (4e-2 rel err - over the
2e-2 gate - and slower: 256-col LDWEIGHTS doesn't pipeline under DR MMs).
"""

import re
from contextlib import ExitStack

import ml_dtypes
import numpy as np

import concourse.bass as bass
import concourse.tile as tile
from concourse import mybir
from concourse.bass_utils import run_bass_kernel_spmd
from bass_rust import ScopedClock, VectorClock

dt = mybir.dt
BF16 = ml_dtypes.bfloat16
FP8 = ml_dtypes.float8_e4m3   # TRN float8e4 (IEEE e4m3, max 240)

# fp8e4 DoubleRow output projection: measured 4.0e-2 rel err (over the 2e-2
# gate; DVE fp32->fp8 cast noise is ~2x the round-to-nearest estimate) AND
# slower (285us vs 245us: DR matmuls don't pipeline with their 256-col
# LDWEIGHTS here). Keep False.
FP8_WO = False

B, L, D = 2, 2048, 2048
H, KVH, HD = 16, 4, 128
G = H // KVH          # 4 query heads per kv head (= per core)
GD = G * HD           # 512: per-core q-head feature dim
THETA = 10000.0
SCALE = HD ** -0.5
NLT = L // 128        # 16 l-tiles
NDT = D // 128        # 16 d-tiles
NLC = L // 512        # 4 l-chunks

# packed input column offsets (bf16 columns of the [128, NCOLS] input),
# ordered by first use: the small K/V weights land first so the K/V
# projections run while the big wq block is still streaming in
OWK = 0                   # 16 tiles x [128, 128]
OWV = OWK + NDT * 128     # 16 tiles x [128, 128]
OX0 = OWV + NDT * 128     # x chunk 0: 16 tiles x [128, 512]
OWQ = OX0 + NDT * 512     # 16 tiles x [128, 512]  (wq, scale folded in)
OCOS = OWQ + NDT * 512    # [128, 2048]  (rope tables after wq: first
OSIN = OCOS + L           # consumer is the K eviction, not a matmul chain,
OTRI = OSIN + L           # and phase A's end is pinned by wq arrival)
OX123 = OTRI + 128        # x chunks 1-3: 3 x 16 tiles x [128, 512]
OWO = OX123 + 3 * NDT * 512  # 4 tiles x [128, 2048]
NCOLS = OWO + G * D


def _patch_tile_drain():
    """walrus in this container rejects multi-wait instructions on the SP
    queue; split the TileContext exit drain into one drain per proc."""
    def _drain_and_barrier_split(self, tick_clock, wait_clock):
        ticks = [int(s) for s in re.findall(r"\d+", str(tick_clock.global_clock))]
        for proc, t in enumerate(ticks):
            if t <= 0:
                continue
            vc = VectorClock()
            vc.require_at_least(proc, t)
            d = self.nc.sync.drain()
            wait_clock.add_sem_waits(d.ins, ScopedClock({None: vc}))
        self.nc.all_engine_barrier()
        assert self.sems is not None
        popped = self.nc._tile_sem_poison_stack.pop()
        assert popped is self._sem_poison
        self.nc.clear_and_free_semaphores(list(self.sems.allocated().values()))
        self.nc.all_engine_barrier()

    tile.TileContext._drain_and_barrier = _drain_and_barrier_split


def _split_multi_waits(nc):
    """This walrus build supports one sem-wait command per instruction; hoist
    excess waits onto same-engine NoOps inserted immediately before."""
    uid = 0
    for fn in nc.m.functions:
        for bb in fn.blocks:
            out = []
            for inst in bb.instructions:
                si = inst.sync_info
                if si is not None and si.on_wait and len(si.on_wait) > 1:
                    for w in si.on_wait[:-1]:
                        nop = mybir.InstNoOp(name=f"waitsplit-{uid}", ins=[], outs=[])
                        uid += 1
                        nop.engine = inst.engine
                        nop.sync_info = mybir.SyncInfo(on_wait=[w], on_update=[])
                        out.append(nop)
                    inst.sync_info = mybir.SyncInfo(
                        on_wait=[si.on_wait[-1]], on_update=si.on_update)
                out.append(inst)
            bb.instructions[:] = out


def _build_program():
    _patch_tile_drain()
    nc = bass.Bass("TRN2", target_bir_lowering=False, debug=False)

    inpack = nc.dram_tensor("inpack", [128, NCOLS], dt.bfloat16, kind="ExternalInput").ap()
    outT = nc.dram_tensor("outT", [D, L], dt.bfloat16, kind="ExternalOutput").ap()

    with tile.TileContext(nc) as tc:
        with ExitStack() as ctx:
            persist = ctx.enter_context(tc.tile_pool(name="persist", bufs=1))

            # --- persistent SBUF residents ---
            wkv = persist.tile([128, 2 * NDT * 128], dt.bfloat16, tag="wkv", name="wkv")
            trig = persist.tile([128, 2 * L + 128], dt.bfloat16, tag="trig", name="trig")
            wqpack = persist.tile([128, NDT * 512], dt.bfloat16, tag="wqpack", name="wqpack")
            xpack = persist.tile([128, NLC * NDT * 512], dt.bfloat16, tag="xpack", name="xpack")
            wo_dt = dt.float8e4 if FP8_WO else dt.bfloat16
            wo_cols = G * D // 2 if FP8_WO else G * D
            wopack = persist.tile([128, wo_cols * (2 if FP8_WO else 1)], wo_dt,
                                  tag="wopack", name="wopack")
            ones_sb = persist.tile([128, 128], dt.bfloat16, tag="ones", name="ones")
            qt_sb = [persist.tile([HD, L], dt.bfloat16, tag=f"qt{h}", name=f"qt{h}") for h in range(G)]
            kt_sb = persist.tile([HD, L], dt.bfloat16, tag="kt", name="kt")
            vpack = persist.tile([128, L], dt.bfloat16, tag="vpack", name="vpack")
            if FP8_WO:
                # two head-pair tiles: [head 2*g2 | head 2*g2+1] along columns
                otp = [persist.tile([128, 2 * L], dt.float8e4, tag=f"otp{g2}", name=f"otp{g2}")
                       for g2 in range(2)]
            else:
                ot_sb = [persist.tile([HD, L], dt.bfloat16, tag=f"ot{h}", name=f"ot{h}")
                         for h in range(G)]

            # --- input DMAs, ordered by first use (single queue runs them
            # FIFO): wk/wv -> x0 (2 halves) -> rope/tri -> wq -> x1..x3 -> wo
            nc.sync.dma_start(out=wkv, in_=inpack[:, OWK:OX0])
            for q in range(4):      # x0 in quarters: smoother chain pacing
                nc.sync.dma_start(
                    out=xpack[:, q * 2048:(q + 1) * 2048],
                    in_=inpack[:, OX0 + q * 2048:OX0 + (q + 1) * 2048])
            nc.sync.dma_start(out=wqpack, in_=inpack[:, OWQ:OCOS])
            nc.sync.dma_start(out=trig, in_=inpack[:, OCOS:OX123])
            for lc in range(1, NLC):
                nc.sync.dma_start(
                    out=xpack[:, lc * 8192:(lc + 1) * 8192],
                    in_=inpack[:, OX123 + (lc - 1) * 8192:OX123 + lc * 8192])
            if FP8_WO:
                # wo8 bytes ride in the bf16 inpack (half the columns)
                nc.sync.dma_start(
                    out=wopack, in_=inpack[:, OWO:OWO + G * D // 2].bitcast(dt.float8e4))
            else:
                nc.sync.dma_start(out=wopack, in_=inpack[:, OWO:NCOLS])
            nc.vector.memset(ones_sb, 1.0)

            cos_sb = trig[:, 0:L]
            sin_sb = trig[:, L:2 * L]
            tri_sb = trig[:, 2 * L:2 * L + 128]
            wk_off = 0            # wk tiles at wkv[:, i*128:...]
            wv_off = OWV - OWK

            def xsl(lc, i):          # moving x tile [128, 512]
                return xpack[:, lc * 8192 + i * 512: lc * 8192 + (i + 1) * 512]

            # SBUF working pools shared by phase A-embedded B0 and phase B/C
            # (deep rings: SBUF is cheap and relaxed WAR edges decouple the
            # DVE/ACT queues from the PE at head boundaries)
            ptp = ctx.enter_context(tc.tile_pool(name="pt", bufs=4))
            accp = ctx.enter_context(tc.tile_pool(name="acc", bufs=3))
            smp = ctx.enter_context(tc.tile_pool(name="sm", bufs=3))
            evp = ctx.enter_context(tc.tile_pool(name="ev", bufs=8))

            # ---- shared attention emit helpers (used for c=0 inside phase A
            # and c=1..3 in the main loop) ----
            def emit_pair(c, bi, qs, psS_p):
                jts = [2 * bi, 2 * bi + 1]
                offs = [(jt - 4 * c) * 128 if jt >= 4 * c else 0 for jt in jts]
                ps = psS_p.tile([128, 1024], dt.float32, tag="psS", name="ps")
                pt = ptp.tile([128, 1024], dt.bfloat16, tag="pt", name="pt")
                for k, (jt, off) in enumerate(zip(jts, offs)):
                    nc.tensor.matmul(
                        ps[:, k * 512 + off:(k + 1) * 512],
                        kt_sb[:, jt * 128:(jt + 1) * 128],
                        qs[:, off:], start=True, stop=True)
                # one exp per pair; masked gap regions get exp'd too (pairs
                # never straddle the diagonal: 4c is even) but are never read
                nc.scalar.activation(
                    pt[:, offs[0]:], ps[:, offs[0]:],
                    mybir.ActivationFunctionType.Exp)
                for k, (jt, off) in enumerate(zip(jts, offs)):
                    if jt >= 4 * c:
                        blk = pt[:, k * 512 + off:k * 512 + off + 128]
                        nc.vector.tensor_tensor(blk, blk, tri_sb, mybir.AluOpType.mult)
                return pt, jts, offs

            def emit_av(c, bi, njt, pair, po, acc):
                pt, jts, offs = pair
                last_bi = bi == njt // 2 - 1
                for k, (jt, off) in enumerate(zip(jts, offs)):
                    pk = pt[:, k * 512 + off:(k + 1) * 512]
                    nc.tensor.matmul(
                        po[:, off:], vpack[:, jt * 128:(jt + 1) * 128], pk,
                        start=(bi == 0 and k == 0), stop=(last_bi and k == 1))
                    if bi == 0 and k == 0:
                        nc.vector.tensor_copy(acc, pk)
                    else:
                        nc.vector.tensor_tensor(
                            acc[:, off:], acc[:, off:], pk, mybir.AluOpType.add)

            def emit_fin(c, h, po, acc, psX_p):
                pr = psX_p.tile([128, 512], dt.float32, tag="psX", name="pr")
                nc.tensor.matmul(pr, ones_sb, acc, start=True, stop=True)
                lnr = smp.tile([128, 512], dt.float32, tag="lnr", name="lnr")
                nc.scalar.activation(lnr, pr, mybir.ActivationFunctionType.Ln)
                rcp = smp.tile([128, 512], dt.float32, tag="rcp", name="rcp")
                nc.scalar.activation(rcp, lnr, mybir.ActivationFunctionType.Exp, scale=-1.0)
                if FP8_WO:
                    odst = otp[h // 2][:, (h % 2) * L + c * 512:(h % 2) * L + (c + 1) * 512]
                else:
                    odst = ot_sb[h][:, c * 512:(c + 1) * 512]
                nc.vector.tensor_tensor(odst, po, rcp, mybir.AluOpType.mult)

            # ---------------- Phase A: projections + rope, with chunk-0
            # attention interleaved piecewise between projection chains ----
            with ExitStack() as ctxA:
                # psA bufs=3 so the K eviction (which waits on the late-
                # arriving rope tables) can hold its PSUM slot without
                # stalling the following Q chains; psV drops to 1 buf (V
                # evictions have ~22us of slack between chunks)
                psA = ctxA.enter_context(tc.tile_pool(name="psA", bufs=3, space="PSUM"))
                psB0s = ctxA.enter_context(tc.tile_pool(name="psB0s", bufs=1, space="PSUM"))
                psB0o = ctxA.enter_context(tc.tile_pool(name="psB0o", bufs=1, space="PSUM"))
                psB0x = ctxA.enter_context(tc.tile_pool(name="psB0x", bufs=1, space="PSUM"))
                ropep = ctxA.enter_context(tc.tile_pool(name="rope", bufs=2))

                def rope_evict(ps, dst_slice, lc):
                    cs = cos_sb[:, lc * 512:(lc + 1) * 512]
                    sn = sin_sb[:, lc * 512:(lc + 1) * 512]
                    swp = ropep.tile([128, 512], dt.bfloat16, tag="swp", name="swp")
                    nc.scalar.copy(swp[0:64, :], ps[64:128, :])
                    nc.scalar.copy(swp[64:128, :], ps[0:64, :])
                    t1 = ropep.tile([128, 512], dt.bfloat16, tag="t1", name="t1")
                    t2 = ropep.tile([128, 512], dt.bfloat16, tag="t2", name="t2")
                    nc.vector.tensor_tensor(t1, swp, sn, mybir.AluOpType.mult)
                    nc.vector.tensor_tensor(t2, ps, cs, mybir.AluOpType.mult)
                    nc.vector.tensor_tensor(dst_slice, t1, t2, mybir.AluOpType.add)

                def b0_head_gen(h):
                    """Chunk-0 attention for head h in 3 pieces (yield = piece
                    boundary); runs on A-phase PSUM pools."""
                    qs = qt_sb[h][:, 0:512]
                    po = psB0o.tile([128, 512], dt.float32, tag="psO", name="po")
                    acc = accp.tile([128, 512], dt.bfloat16, tag="acc", name="acc")
                    pairs = []
                    for bi in range(2):
                        pairs.append(emit_pair(0, bi, qs, psB0s))
                        yield
                        emit_av(0, bi, 4, pairs[bi], po, acc)
                    emit_fin(0, h, po, acc, psB0x)

                def _step(g):
                    try:
                        next(g)
                    except StopIteration:
                        pass

                pending = []

                def pump():
                    if pending:
                        pending.pop(0)()

                def q_chain(lc, ot):
                    ps = psA.tile([128, 512], dt.float32, tag="psA", name="psA")
                    for i in range(NDT):
                        nc.tensor.matmul(
                            ps, wqpack[:, i * 512 + ot * 128:i * 512 + (ot + 1) * 128],
                            xsl(lc, i), start=(i == 0), stop=(i == NDT - 1))
                    rope_evict(ps, qt_sb[ot][:, lc * 512:(lc + 1) * 512], lc)

                def k_chain(lc):
                    ps = psA.tile([128, 512], dt.float32, tag="psA", name="psA")
                    for i in range(NDT):
                        nc.tensor.matmul(
                            ps, wkv[:, wk_off + i * 128:wk_off + (i + 1) * 128],
                            xsl(lc, i), start=(i == 0), stop=(i == NDT - 1))
                    rope_evict(ps, kt_sb[:, lc * 512:(lc + 1) * 512], lc)

                def v_chain(lc):
                    pv = psA.tile([128, 512], dt.float32, tag="psV", bufs=1, name="psV")
                    for ls in range(4):
                        for i in range(NDT):
                            nc.tensor.matmul(
                                pv[:, ls * 128:(ls + 1) * 128],
                                xpack[:, lc * 8192 + i * 512 + ls * 128:
                                      lc * 8192 + i * 512 + (ls + 1) * 128],
                                wkv[:, wv_off + i * 128:wv_off + (i + 1) * 128],
                                start=(i == 0), stop=(i == NDT - 1))
                    nc.vector.tensor_copy(vpack[:, lc * 512:(lc + 1) * 512], pv)

                for lc in range(NLC):
                    if lc == 2:
                        for h in range(G):
                            g = b0_head_gen(h)
                            pending.extend([lambda g=g: _step(g)] * 3)
                    if lc == 0:     # K/V first: wq is still streaming in
                        chains = [lambda: k_chain(0), lambda: v_chain(0)] + [
                            (lambda ot=ot: q_chain(0, ot)) for ot in range(G)]
                    else:
                        chains = [(lambda ot=ot: q_chain(lc, ot)) for ot in range(G)] + [
                            lambda lc=lc: k_chain(lc), lambda lc=lc: v_chain(lc)]
                    for ch in chains:
                        pump()
                        ch()
                assert not pending

            # ---------------- Phase B+C: attention + out-projection ----------------
            with ExitStack() as ctxB:
                psS = ctxB.enter_context(tc.tile_pool(name="psS", bufs=2, space="PSUM"))
                psO = ctxB.enter_context(tc.tile_pool(name="psO", bufs=2, space="PSUM"))
                psX = ctxB.enter_context(tc.tile_pool(name="psX", bufs=2, space="PSUM"))

                def out_proj_group(lc, ets, borrow_psS=False):
                    """Output projection for l-chunk lc, e-tiles ets (even
                    count). Evicts land in [128,1024] pair tiles; one DMA per
                    et-pair (halves the Sync descriptor cost). borrow_psS
                    alternates pw tiles into the idle psS slots (final batch
                    only, when no more S-pairs need them) for a 4-deep ring."""
                    ev2 = None
                    for n, et in enumerate(ets):
                        if borrow_psS and n % 2 == 1:
                            pw = psS.tile([128, 512], dt.float32, tag="psS", name="pw")
                        else:
                            pw = psX.tile([128, 512], dt.float32, tag="psX", name="pw")
                        if FP8_WO:
                            for g2 in range(2):
                                lhsT = wopack[:, (g2 * NDT + et) * 256:
                                              (g2 * NDT + et + 1) * 256].rearrange(
                                    "p (j m) -> p j m", j=2)
                                rhs = otp[g2].rearrange("p (j l) -> p j l", j=2)[
                                    :, :, lc * 512:(lc + 1) * 512]
                                nc.tensor.matmul(
                                    pw, lhsT, rhs, start=(g2 == 0), stop=(g2 == 1),
                                    perf_mode=mybir.MatmulPerfMode.DoubleRow)
                        else:
                            for g in range(G):
                                nc.tensor.matmul(
                                    pw, wopack[:, g * D + et * 128:g * D + (et + 1) * 128],
                                    ot_sb[g][:, lc * 512:(lc + 1) * 512],
                                    start=(g == 0), stop=(g == G - 1))
                        if n % 2 == 0:
                            ev2 = evp.tile([128, 1024], dt.bfloat16, tag="ev", name="ev")
                            et0 = et
                            nc.vector.tensor_copy(ev2[:, 0:512], pw)
                        else:
                            nc.scalar.copy(ev2[:, 512:1024], pw)
                            dst = outT[et0 * 128:(et0 + 2) * 128,
                                       lc * 512:(lc + 1) * 512].rearrange(
                                "(k p) c -> p k c", k=2)
                            nc.sync.dma_start(
                                out=dst, in_=ev2.rearrange("p (k c) -> p k c", k=2))

                for c in range(1, NLC):
                    njt = 4 * (c + 1)
                    for h in range(G):
                        qs = qt_sb[h][:, c * 512:(c + 1) * 512]
                        po = psO.tile([128, 512], dt.float32, tag="psO", name="po")
                        acc = accp.tile([128, 512], dt.bfloat16, tag="acc", name="acc")
                        for bi in range(njt // 2):
                            pair = emit_pair(c, bi, qs, psS)
                            emit_av(c, bi, njt, pair, po, acc)
                        # interleave prev chunk's out-projection BEFORE the
                        # rowsum matmul: the in-order PE queue chews these
                        # ready MMs while the DVE acc chain catches up
                        out_proj_group(c - 1, range(4 * h, 4 * h + 4))
                        emit_fin(c, h, po, acc, psX)
                    if c == NLC - 1:    # last chunk's out-projection runs at the end
                        out_proj_group(c, range(NDT), borrow_psS=True)
    _split_multi_waits(nc)
    return nc


_PROG = None


def _rope_tables():
    inv_freq = 1.0 / (THETA ** (np.arange(0, HD, 2, dtype=np.float32) / HD))
    t = np.arange(L, dtype=np.float32)
    freqs = np.outer(t, inv_freq)
    emb = np.concatenate([freqs, freqs], axis=-1)      # [L, HD]
    cos = np.cos(emb).T.copy()                         # [HD, L]
    sin = np.sin(emb).T.copy()
    sin_eff = sin.copy()
    sin_eff[:64] = -sin_eff[:64]                       # dest-indexed rotate_half sign
    return cos, sin_eff


def _tiles_to_cols(a, tp):
    """[tp*128, C] -> [128, tp*C]: row-tile i becomes column block i."""
    tpn = a.shape[0] // 128
    return np.ascontiguousarray(
        a.reshape(tpn, 128, a.shape[1]).transpose(1, 0, 2).reshape(128, -1))


def _prepare_in_maps(x, Wq, Wk, Wv, Wo):
    cos, sin_eff = _rope_tables()
    bfc = lambda a: np.ascontiguousarray(a).astype(BF16)
    x, Wq, Wk, Wv, Wo = (np.asarray(a, dtype=np.float32) for a in (x, Wq, Wk, Wv, Wo))

    tri = np.tril(np.ones((128, 128), dtype=np.float32)).T  # 1 where pj <= fq

    xb = []
    for b in range(B):
        xT = x[b].T                                     # [D, L]
        chunks = [_tiles_to_cols(xT[:, lc * 512:(lc + 1) * 512], 16) for lc in range(NLC)]
        xb.append(np.concatenate(chunks, axis=1))       # [128, 32768]

    in_maps = []
    for c in range(8):
        b, g = c // 4, c % 4
        wqT = (Wq[g * GD:(g + 1) * GD, :] * SCALE).T    # [D, GD] scale folded
        wkT = Wk[g * HD:(g + 1) * HD, :].T              # [D, HD]
        wvT = Wv[g * HD:(g + 1) * HD, :].T
        woT = Wo[:, g * GD:(g + 1) * GD].T              # [GD, D]
        if FP8_WO:
            # DoubleRow layout: col = (g2*16+et)*256 + j*128 + m holds
            # woT[g2*256 + j*128 + p, et*128 + m]; fp8 bytes ride as bf16
            wo8 = np.ascontiguousarray(
                woT.reshape(2, 2, 128, NDT, 128).transpose(2, 0, 3, 1, 4)
                .reshape(128, G * D)).astype(FP8)
            wo_cols = np.concatenate([
                wo8.view(np.uint8).view(BF16),          # [128, 4096]
                np.zeros((128, G * D // 2), dtype=BF16)], axis=1)
        else:
            wo_cols = _tiles_to_cols(woT, 4)
        pack = np.concatenate([
            bfc(_tiles_to_cols(wkT, 16)),               # 2048
            bfc(_tiles_to_cols(wvT, 16)),               # 2048
            bfc(xb[b][:, 0:8192]),                      # x chunk 0
            bfc(_tiles_to_cols(wqT, 16)),               # 8192
            bfc(cos), bfc(sin_eff), bfc(tri),           # 2048+2048+128
            bfc(xb[b][:, 8192:]),                       # x chunks 1-3
            wo_cols.astype(BF16) if wo_cols.dtype != BF16 else wo_cols,
        ], axis=1)
        assert pack.shape == (128, NCOLS), pack.shape
        in_maps.append({"inpack": pack})
    return in_maps


def _run(in_maps, **kwargs):
    global _PROG
    if _PROG is None:
        _PROG = _build_program()
    return run_bass_kernel_spmd(_PROG, in_maps, list(range(8)), **kwargs)


def _gather(res):
    out = np.zeros((B, L, D), dtype=np.float32)
    for c in range(8):
        b = c // 4
        out[b] += res.results[c]["outT"].astype(np.float32).T
    return out


def kernel(x, Wq, Wk, Wv, Wo):
    return _gather(_run(_prepare_in_maps(x, Wq, Wk, Wv, Wo)))
